# revision 8
# baseline (speedup 1.0000x reference)
"""GQA attention (B=2, S=2048, D=2048, 32 Q heads / 8 KV heads, HD=64, RoPE,
causal) on 8 TRN2 NeuronCores.

Sharding: tensor-parallel over heads. Core c owns q heads [4c, 4c+4) and kv
head c (GQA groups align exactly with 8 cores); both batches replicated.

Fully software-pipelined single stream per core:
  - batch-0 projections (qkv + RoPE, transposed layout) run PE-dense;
  - batch-0 attention (s^T layout, merged [128,1024] exp on ACT, ones-row
    softmax denominators) is exp-latency-bound on the scalar engine, so
    batch-1 projection matmul chunks are interleaved into the PE stream to
    fill the gaps; batch-1 attention interleaves the batch-0 wo matmuls the
    same way.
  - attention output ships per 512-query block: 8 AllGather segments
    (bf16, 0.25MB/rank) fired as soon as each block is normalized, with the
    qsub blocks processed high-to-low so the final (smallest) block's
    gather+wo tail is minimal.
  - PSUM: scores 4 banks (2 bufs x [128,1024]), PV accumulators 2 banks,
    shared proj/wo/transpose pool 2 banks = 8.
All HBM->SBUF loads use host-prearranged per-partition-contiguous layouts.
Compute dtype: bf16 matmul operands, fp32 PSUM accumulation, fp32 softmax.
"""

import numpy as np

B, S, D = 2, 2048, 2048
H, KVH, HD = 32, 8, 64
NCORES = 8
BS = B * S            # 4096
NHL = H // NCORES     # 4 q heads per core
MQ = NHL * HD         # 256 q columns per core
SBLK = 512
NSBLK = BS // SBLK    # 8
DC = D // 128         # 16 contraction chunks
NKCH = S // 128       # 16 key chunks per batch
NQS = S // SBLK       # 4 query blocks per batch


def build_graph(tc, out_ap, xT, wqEO, kvw, woS, c4, s4):
    """Build the per-core SPMD graph. All args are DRAM access patterns."""
    import concourse.mybir as mybir
    from concourse.masks import make_identity

    nc = tc.nc
    F32 = mybir.dt.float32
    BF16 = mybir.dt.bfloat16
    Alu = mybir.AluOpType
    Act = mybir.ActivationFunctionType
    TT = nc.vector.tensor_tensor
    CP = nc.vector.tensor_copy

    with tc.tile_pool(name="const", bufs=1) as constp, \
         tc.tile_pool(name="persist", bufs=1) as pers, \
         tc.tile_pool(name="dram", bufs=1, space="DRAM") as dramp:

        ident = constp.tile([128, 128], F32)
        make_identity(nc, ident[:])
        tri01f = constp.tile([128, 128], F32)
        nc.gpsimd.memset(tri01f[:], 1.0)
        nc.gpsimd.affine_select(
            out=tri01f[:], in_=tri01f[:], compare_op=Alu.is_ge, fill=0.0,
            base=0, channel_multiplier=-1, pattern=[[1, 128]])
        tri01 = constp.tile([128, 128], BF16)
        CP(tri01[:], tri01f[:])

        # weights/trig DMAs, ordered so the first projection chunk can start
        # ~2us in: wq g0, x(sb0), wq g1-3, kvw, trig
        wq_sb = constp.tile([128, DC, MQ], BF16)
        wqr = wqEO.rearrange("p (dc m) -> p dc m", dc=DC)
        nc.sync.dma_start(wq_sb[:, 0:4, :], wqr[:, 0:4, :])

        xTr = xT.rearrange("p (sb dc s) -> p sb dc s", sb=NSBLK, dc=DC)

        with tc.tile_pool(name="trig", bufs=1) as trigp, \
             tc.tile_pool(name="xtp", bufs=6) as xtp, \
             tc.tile_pool(name="ropep", bufs=2) as rp, \
             tc.tile_pool(name="ptp", bufs=2) as ptp, \
             tc.tile_pool(name="recp", bufs=2) as recp, \
             tc.tile_pool(name="wop", bufs=1) as wop, \
             tc.tile_pool(name="attsp", bufs=3) as attsp, \
             tc.tile_pool(name="outsp", bufs=2) as outsp:
            psA = psS = psO = psX = None  # PSUM pools, scoped in the driver

            def xt_dmas(sb):
                xts = []
                for g in range(4):
                    xt = xtp.tile([128, 4, SBLK], BF16, tag="xt", name=f"xt{g}")
                    nc.sync.dma_start(xt[:], xTr[:, sb, 4 * g:4 * g + 4, :])
                    xts.append(xt)
                return xts

            xts0 = xt_dmas(0)
            for g in range(1, 4):
                nc.sync.dma_start(wq_sb[:, 4 * g:4 * g + 4, :],
                                  wqr[:, 4 * g:4 * g + 4, :])
            kvw_sb = constp.tile([128, DC, 128], BF16)
            nc.sync.dma_start(kvw_sb[:], kvw.rearrange("p (dc m) -> p dc m", dc=DC))
            c4_sb = trigp.tile([128, S], F32)
            nc.sync.dma_start(c4_sb[:], c4[:])
            s4_sb = trigp.tile([128, S], F32)
            nc.sync.dma_start(s4_sb[:], s4[:])
            wo_sb = wop.tile([128, DC, MQ], BF16)
            nc.sync.dma_start(wo_sb[:], woS.rearrange("p (dc m) -> p dc m", dc=DC))

            qt0 = pers.tile([128, BS], BF16)   # heads 0,1 (rows [0:64], [64:128])
            qt1 = pers.tile([128, BS], BF16)   # heads 2,3
            kT2 = pers.tile([128, BS], BF16)   # kT duplicated at base 0 and 64
            v1 = pers.tile([128, B * NKCH, 128], BF16)  # [v | ones] per chunk
            attnT0 = pers.tile([128, BS], BF16)
            attnT1 = pers.tile([128, BS], BF16)
            qts = [qt0, qt1]
            attnTs = [attnT0, attnT1]
            nc.gpsimd.memset(v1[:, :, 64:128], 1.0)

            def proj_sb(sb, xts, on_act, P):
                """Generator: projection+RoPE for one 512-col block of x^T.
                Yields ~1us-of-PE chunks. on_act: route copies to the scalar
                engine (idle outside attention) instead of DVE. P(tag) is the
                PSUM allocator (dedicated pool for b0, shared psX for b1)."""
                scol = slice(sb * SBLK, (sb + 1) * SBLK)
                pbeg = (sb % NQS) * SBLK
                qE_p = P("qE")
                for dc in range(DC):
                    nc.tensor.matmul(qE_p[:], wq_sb[:, dc, 0:128],
                                     xts[dc // 4][:, dc % 4],
                                     start=(dc == 0), stop=(dc == DC - 1))
                    if dc % 4 == 3:
                        yield
                qO_p = P("qO")
                for dc in range(DC):
                    nc.tensor.matmul(qO_p[:], wq_sb[:, dc, 128:256],
                                     xts[dc // 4][:, dc % 4],
                                     start=(dc == 0), stop=(dc == DC - 1))
                    if dc % 4 == 3:
                        yield
                c_ = c4_sb[:, pbeg:pbeg + SBLK]
                s_ = s4_sb[:, pbeg:pbeg + SBLK]
                # q RoPE: qE_p rows = 4 heads x 32 even comps, qO_p odd comps
                m1 = rp.tile([128, SBLK], F32, tag="m1")
                TT(m1[:], qE_p[:], c_, Alu.mult)
                m2 = rp.tile([128, SBLK], F32, tag="m2")
                TT(m2[:], qO_p[:], s_, Alu.mult)
                m3 = rp.tile([128, SBLK], F32, tag="m3")
                TT(m3[:], qO_p[:], c_, Alu.mult)
                m4 = rp.tile([128, SBLK], F32, tag="m4")
                TT(m4[:], qE_p[:], s_, Alu.mult)
                oE = rp.tile([128, SBLK], BF16, tag="oE")
                TT(oE[:], m1[:], m2[:], Alu.subtract)
                oO = rp.tile([128, SBLK], BF16, tag="oO")
                TT(oO[:], m3[:], m4[:], Alu.add)
                kv_p = P("kv")
                for dc in range(DC):
                    nc.tensor.matmul(kv_p[:], kvw_sb[:, dc, :],
                                     xts[dc // 4][:, dc % 4],
                                     start=(dc == 0), stop=(dc == DC - 1))
                    if dc % 4 == 3:
                        yield
                cp = nc.scalar.copy if on_act else CP
                # v first: evacuating vT promptly unblocks this PSUM slot for
                # the next block's chain (it's the slot-gating reader)
                vtw = rp.tile([64, SBLK], F32, tag="vtw")
                cp(vtw[:], kv_p[64:128, :])
                # k RoPE: kv_p rows [0:32]=kE, [32:64]=kO, [64:128]=vT
                a1 = rp.tile([32, SBLK], F32, tag="a1")
                TT(a1[:], kv_p[0:32, :], c_[0:32, :], Alu.mult)
                b1 = rp.tile([32, SBLK], F32, tag="b1")
                TT(b1[:], kv_p[32:64, :], s_[0:32, :], Alu.mult)
                a2 = rp.tile([32, SBLK], F32, tag="a2")
                TT(a2[:], kv_p[32:64, :], c_[0:32, :], Alu.mult)
                b2 = rp.tile([32, SBLK], F32, tag="b2")
                TT(b2[:], kv_p[0:32, :], s_[0:32, :], Alu.mult)
                TT(kT2[0:32, scol], a1[:], b1[:], Alu.subtract)
                TT(kT2[32:64, scol], a2[:], b2[:], Alu.add)
                yield
                tpx = P("tp")
                for q in range(4):
                    nc.tensor.transpose(tpx[:, 64 * q:64 * q + 64],
                                        vtw[:, q * 128:(q + 1) * 128],
                                        ident[0:64, 0:64])
                ch = sb * 4
                CP(v1[:, ch:ch + 4, 0:64],
                   tpx[:, 0:256].rearrange("p (c f) -> p c f", c=4))
                yield
                for h in range(NHL):
                    t, j = h // 2, h % 2
                    cp(qts[t][64 * j:64 * j + 32, scol], oE[32 * h:32 * h + 32, :])
                    cp(qts[t][64 * j + 32:64 * j + 64, scol], oO[32 * h:32 * h + 32, :])
                yield
                cp(kT2[64:96, scol], kT2[0:32, scol])
                cp(kT2[96:128, scol], kT2[32:64, scol])
                yield

            def proj_stream(sbs, on_act, P):
                """Chain proj generators with 1-block xt DMA lookahead."""
                pending = {sb: None for sb in sbs}
                pending[sbs[0]] = xts0 if sbs[0] == 0 else xt_dmas(sbs[0])
                for idx, sb in enumerate(sbs):
                    if idx + 1 < len(sbs):
                        pending[sbs[idx + 1]] = xt_dmas(sbs[idx + 1])
                    yield from proj_sb(sb, pending[sb], on_act, P)

            # ---- attention output segments: one per (b, qsub); the last
            # seg is gathered per head-pair so its t=0 half ships while t=1
            # attention still runs ----
            attnT_loc = [dramp.tile([MQ, SBLK], BF16, name=f"attnT_loc{i}")
                         for i in range(8)]
            attnT_all = [dramp.tile([D, SBLK], BF16, addr_space="Shared",
                                    name=f"attnT_all{i}") for i in range(8)]
            attnT_loc7 = [dramp.tile([128, SBLK], BF16, name=f"attnT_loc7{t}")
                          for t in range(2)]
            attnT_all7 = [dramp.tile([D // 2, SBLK], BF16, addr_space="Shared",
                                     name=f"attnT_all7{t}") for t in range(2)]

            def wo_dmas(seg):
                attr = attnT_all[seg].rearrange("(dc p) s -> p dc s", p=128)
                atts = []
                for hf in range(2):
                    at = attsp.tile([128, DC // 2, SBLK], BF16, tag="att",
                                    name=f"att{hf}")
                    nc.sync.dma_start(at[:], attr[:, 8 * hf:8 * hf + 8, :])
                    atts.append(at)
                return atts

            def wo_seg(seg, atts):
                """Generator: wo matmuls for one gathered 512-col segment."""
                b, qs = seg // NQS, seg % NQS
                for mc in range(2):
                    wp = psX.tile([128, SBLK], F32, tag="x", name="wp")
                    for dc in range(DC):
                        nc.tensor.matmul(
                            wp[:], wo_sb[:, dc, mc * 128:(mc + 1) * 128],
                            atts[dc // 8][:, dc % 8, :],
                            start=(dc == 0), stop=(dc == DC - 1))
                        if dc % 4 == 3:
                            yield
                    ot = outsp.tile([128, SBLK], F32, tag="ot")
                    CP(ot[:], wp[:])
                    nc.sync.dma_start(
                        out_ap[mc * 128:(mc + 1) * 128,
                               b * S + qs * SBLK:b * S + (qs + 1) * SBLK],
                        ot[:])
                    yield

            gathered_at = {}   # seg -> fill counter when its gather emitted
            fills = [0]        # shared fill-slot counter
            MARGIN = 16        # fill slots (~20us) for the gather to execute

            def wo_att7_load(t):
                attr = attnT_all7[t].rearrange("(c p) s -> p c s", p=128)
                at = attsp.tile([128, NCORES, SBLK], BF16, tag="att",
                                name=f"att7{t}")
                nc.sync.dma_start(at[:], attr[:])
                return at

            def wo_seg7_half(t, at, wps):
                """One head-pair half of the final wo segment. t=0 opens the
                PSUM accumulation (runs while the t=1 gather is in flight);
                t=1 closes it and ships the output."""
                b, qs = 1, 3
                for mc in range(2):
                    if t == 0:
                        wps.append(psX.tile([128, SBLK], F32, tag="x",
                                            name="wp"))
                    wp = wps[mc]
                    for c in range(NCORES):
                        nc.tensor.matmul(
                            wp[:], wo_sb[:, 2 * c + t, mc * 128:(mc + 1) * 128],
                            at[:, c, :],
                            start=(t == 0 and c == 0),
                            stop=(t == 1 and c == NCORES - 1))
                    if t == 1:
                        ot = outsp.tile([128, SBLK], F32, tag="ot")
                        CP(ot[:], wp[:])
                        nc.sync.dma_start(
                            out_ap[mc * 128:(mc + 1) * 128,
                                   b * S + qs * SBLK:b * S + (qs + 1) * SBLK],
                            ot[:])

            def wo_stream(segs):
                """wo segments with att-tile DMA prefetch one seg ahead,
                gated on the seg's gather having been emitted MARGIN fill
                slots ago (so the collective has likely completed)."""
                atts = {}

                def ready(s):
                    return s in gathered_at and \
                        fills[0] >= gathered_at[s] + MARGIN

                def ensure(s):
                    if s is not None and s not in atts and ready(s):
                        atts[s] = wo_dmas(s)

                def gen():
                    for idx, seg in enumerate(segs):
                        nxt = segs[idx + 1] if idx + 1 < len(segs) else None
                        while not ready(seg):
                            yield
                        ensure(seg)
                        for _ in wo_seg(seg, atts.pop(seg)):
                            ensure(nxt)
                            yield
                return gen()

            def fill(gen):
                fills[0] += 1
                if gen is not None:
                    next(gen, None)

            rA, rB = slice(0, 64), slice(64, 128)

            def attn_batch(b, filler, fill_every, qs_order):
                """Attention for batch b, pulling filler chunks into the PE
                stream between iterations."""
                it = 0
                pend = []
                for qs in qs_order:
                    qcg = slice(b * S + qs * 512, b * S + (qs + 1) * 512)
                    for t in range(2):
                        oP = [psO.tile([128, SBLK], F32, tag=f"o{i}",
                                       name=f"o{i}") for i in range(2)]
                        # diagonal chunks first: the PV accumulation then
                        # starts and stops on full-width matmuls, with the
                        # narrowed diag writes in the middle
                        ks = list(range(4 * qs, 4 * qs + 4)) + \
                            list(range(0, 4 * qs))
                        for ki, k in enumerate(ks):
                            diag = (k // 4) == qs
                            es = 128 * (k % 4) if diag else 0
                            full_pv = qs == 0  # all-diag chain: pad with 0s
                            kc = slice(b * S + k * 128, b * S + k * 128 + 128)
                            qc = slice(b * S + qs * 512 + es,
                                       b * S + qs * 512 + 512)
                            sP = psS.tile([128, 1024], F32, tag="s", name="s")
                            nc.tensor.matmul(sP[:, es:512], kT2[rA, kc],
                                             qts[t][rA, qc],
                                             start=True, stop=True)
                            nc.tensor.matmul(sP[:, 512 + es:1024], kT2[rB, kc],
                                             qts[t][rB, qc],
                                             start=True, stop=True)
                            pP = ptp.tile([128, 1024], BF16, tag="p", name="p")
                            if es and full_pv:
                                nc.vector.memset(pP[:, 0:es], 0.0)
                                nc.vector.memset(pP[:, 512:512 + es], 0.0)
                            if es:
                                nc.scalar.activation(
                                    pP[:, es:512], sP[:, es:512],
                                    Act.Exp, scale=0.125)
                                nc.scalar.activation(
                                    pP[:, 512 + es:1024], sP[:, 512 + es:1024],
                                    Act.Exp, scale=0.125)
                            else:
                                nc.scalar.activation(
                                    pP[:], sP[:], Act.Exp, scale=0.125)
                            if diag:
                                # zero the causal triangle (key > q)
                                TT(pP[:, es:es + 128], pP[:, es:es + 128],
                                   tri01[:], Alu.mult)
                                TT(pP[:, 512 + es:512 + es + 128],
                                   pP[:, 512 + es:512 + es + 128],
                                   tri01[:], Alu.mult)
                            it += 1
                            if it % fill_every == 0:
                                fill(filler)
                            if pend:
                                pend.pop()()
                            def pv(pP=pP, k=k, ki=ki,
                                   es=(0 if full_pv else es)):
                                for i in range(2):
                                    nc.tensor.matmul(
                                        oP[i][:, es:512],
                                        v1[:, b * NKCH + k, :],
                                        pP[:, 512 * i + es:512 * i + 512],
                                        start=(ki == 0),
                                        stop=(ki == 4 * qs + 3),
                                        skip_group_check=(es != 0))
                            pend.append(pv)
                        if pend:
                            pend.pop()()
                        # normalize: oP rows [64:128] hold the denominator
                        for i, rows in enumerate((rA, rB)):
                            raw = recp.tile([128, SBLK], F32, tag=f"raw{i}",
                                            name=f"raw{i}")
                            CP(raw[:], oP[i][:])
                            den = recp.tile([64, SBLK], F32, tag=f"den{i}",
                                            name=f"den{i}")
                            CP(den[:], raw[64:128, :])
                            rec = recp.tile([64, SBLK], F32, tag=f"rec{i}",
                                            name=f"rec{i}")
                            nc.vector.reciprocal_approx_fast(rec[:], den[:])
                            TT(attnTs[t][rows, qcg], raw[0:64, :], rec[:],
                               Alu.mult)
                        if b * NQS + qs == 7:
                            nc.gpsimd.dma_start(attnT_loc7[t][:],
                                                attnTs[t][:, qcg])
                            nc.gpsimd.collective_compute(
                                "AllGather", mybir.AluOpType.bypass,
                                replica_groups=[list(range(NCORES))],
                                ins=[attnT_loc7[t].opt()],
                                outs=[attnT_all7[t].opt()])
                            gathered_at[(7, t)] = fills[0]
                        fill(filler)
                    seg = b * NQS + qs
                    if seg < 7:
                        nc.gpsimd.dma_start(attnT_loc[seg][0:128, :],
                                            attnTs[0][:, qcg])
                        nc.gpsimd.dma_start(attnT_loc[seg][128:256, :],
                                            attnTs[1][:, qcg])
                        nc.gpsimd.collective_compute(
                            "AllGather", mybir.AluOpType.bypass,
                            replica_groups=[list(range(NCORES))],
                            ins=[attnT_loc[seg].opt()],
                            outs=[attnT_all[seg].opt()])
                        gathered_at[seg] = fills[0]

            # ---- the pipelined program ----
            def PA(tag):
                return psA.tile([128, SBLK], F32, tag=tag, name=tag)

            def PX(tag):
                return psX.tile([128, SBLK], F32, tag="x", name=tag)

            with tc.tile_pool(name="psA", bufs=2, space="PSUM") as psA:
                projA = proj_stream([0, 1, 2, 3], True, PA)
                for _ in projA:
                    pass
            with tc.tile_pool(name="psS", bufs=2, space="PSUM") as psS, \
                 tc.tile_pool(name="psO", bufs=1, space="PSUM") as psO, \
                 tc.tile_pool(name="psX", bufs=2, space="PSUM") as psX:
                import itertools
                projB = proj_stream([4, 5, 6, 7], False, PX)
                # one unified filler: b1 projections first, then wo segments
                # as their gathers complete -- wo seg 0/1 work lands in the
                # late-b0 fill slots that projB can't cover. Seg order
                # matches gather production order (b1 runs qs [2,0,1,3]).
                woA = wo_stream([0, 1, 2, 3, 6, 4, 5])
                filler = itertools.chain(projB, woA)
                attn_batch(0, filler, 1, [0, 1, 2, 3])
                # qs2 first: its gather (the would-be tail straggler) fires
                # early; qs3's split gather halves are then the only tail work
                attn_batch(1, filler, 1, [2, 0, 1, 3])
                fills[0] = 1 << 30   # drain: no more fill slots, no gating
                for _ in filler:
                    pass
                # final segment: the t=0 half opens the PSUM accumulation as
                # soon as its gather lands, overlapping the t=1 gather wait
                wps = []
                wo_seg7_half(0, wo_att7_load(0), wps)
                wo_seg7_half(1, wo_att7_load(1), wps)


def prep_inputs(x, cos, sin, wq, wk, wv, wo):
    """Host-side layout prep. Returns per-core input dicts (bf16/f32).

    All SBUF-bound tensors are prearranged so that each of the 128 SBUF
    partitions reads one contiguous DRAM chunk (fat DMA descriptors).
    """
    import ml_dtypes
    bf16 = ml_dtypes.bfloat16
    x = np.asarray(x, np.float32)
    cos = np.asarray(cos, np.float32)
    sin = np.asarray(sin, np.float32)
    wq = np.asarray(wq, np.float32)
    wk = np.asarray(wk, np.float32)
    wv = np.asarray(wv, np.float32)
    wo = np.asarray(wo, np.float32)

    xT = np.ascontiguousarray(x.transpose(2, 0, 1).reshape(D, BS)).astype(bf16)
    # [dc*128+p, sb*512+s] -> [p, sb, dc, s] flattened per partition
    xTc = np.ascontiguousarray(
        xT.reshape(DC, 128, NSBLK, SBLK).transpose(1, 2, 0, 3).reshape(128, -1))
    c4 = np.ascontiguousarray(np.tile(cos.T, (4, 1)))          # [128, S] f32
    s4 = np.ascontiguousarray(np.tile(sin.T, (4, 1)))
    eperm = np.array([64 * h + 2 * j for h in range(NHL) for j in range(32)])
    operm = eperm + 1

    def pmajor(w):  # [D, M] -> [128, DC*M] with [p, dc, m] contiguous
        m = w.shape[1]
        return np.ascontiguousarray(
            w.reshape(DC, 128, m).transpose(1, 0, 2).reshape(128, -1)).astype(bf16)

    in_maps = []
    for c in range(NCORES):
        wq_sh = wq[:, MQ * c:MQ * c + MQ]
        wqEO = np.concatenate([wq_sh[:, eperm], wq_sh[:, operm]], axis=1)
        kc = wk[:, HD * c:HD * c + HD]
        vc = wv[:, HD * c:HD * c + HD]
        kvw = np.concatenate([kc[:, 0::2], kc[:, 1::2], vc], axis=1)
        woS = wo[:, MQ * c:MQ * c + MQ]
        in_maps.append({
            "xT": xTc,
            "wqEO": pmajor(wqEO),
            "kvw": pmajor(kvw),
            "woS": pmajor(woS),
            "c4": c4,
            "s4": s4,
        })
    return in_maps


def assemble_output(core_outs):
    """core_outs: list of 8 [256, BS] f32 arrays -> [B, S, D] f32."""
    outT = np.concatenate(core_outs, axis=0)           # [D, BS]
    return np.ascontiguousarray(
        outT.reshape(D, B, S).transpose(1, 2, 0)).astype(np.float32)


_CACHE = {}


def _get_compiled():
    if "nc" in _CACHE:
        return _CACHE["nc"]
    import concourse.mybir as mybir
    import concourse.tile as tile
    from concourse import bacc

    nc = bacc.Bacc("TRN2", target_bir_lowering=False, debug=False,
                   num_devices=NCORES)
    F32 = mybir.dt.float32
    BF16 = mybir.dt.bfloat16
    xT_d = nc.dram_tensor("xT", [128, NSBLK * DC * SBLK], BF16, kind="ExternalInput")
    wq_d = nc.dram_tensor("wqEO", [128, DC * MQ], BF16, kind="ExternalInput")
    kvw_d = nc.dram_tensor("kvw", [128, DC * 128], BF16, kind="ExternalInput")
    wo_d = nc.dram_tensor("woS", [128, DC * MQ], BF16, kind="ExternalInput")
    c4_d = nc.dram_tensor("c4", [128, S], F32, kind="ExternalInput")
    s4_d = nc.dram_tensor("s4", [128, S], F32, kind="ExternalInput")
    out_d = nc.dram_tensor("out", [MQ, BS], F32, kind="ExternalOutput")
    with tile.TileContext(nc) as tc:
        build_graph(tc, out_d.ap(), xT_d.ap(), wq_d.ap(), kvw_d.ap(),
                    wo_d.ap(), c4_d.ap(), s4_d.ap())
    nc.compile()
    _CACHE["nc"] = nc
    return nc


def kernel(x, cos, sin, wq, wk, wv, wo):
    from concourse.bass_utils import run_bass_kernel_spmd
    nc = _get_compiled()
    in_maps = prep_inputs(x, cos, sin, wq, wk, wv, wo)
    res = run_bass_kernel_spmd(nc, in_maps, core_ids=list(range(NCORES)))
    _CACHE["last_results"] = res
    return assemble_output([res.results[c]["out"] for c in range(NCORES)])



# revision 13
# speedup vs baseline: 1.0283x; 1.0283x over previous
"""GQA attention (B=2, S=2048, D=2048, 32 Q heads / 8 KV heads, HD=64, RoPE,
causal) on 8 TRN2 NeuronCores.

Sharding: tensor-parallel over heads. Core c owns q heads [4c, 4c+4) and kv
head c (GQA groups align exactly with 8 cores); both batches replicated.

Fully software-pipelined single stream per core:
  - batch-0 projections (qkv + RoPE, transposed layout) run PE-dense;
  - batch-0 attention (s^T layout, merged [128,1024] exp on ACT, ones-row
    softmax denominators) is exp-latency-bound on the scalar engine, so
    batch-1 projection matmul chunks are interleaved into the PE stream to
    fill the gaps; batch-1 attention interleaves the batch-0 wo matmuls the
    same way.
  - attention output ships per 512-query block: 8 AllGather segments
    (bf16, 0.25MB/rank) fired as soon as each block is normalized, with the
    qsub blocks processed high-to-low so the final (smallest) block's
    gather+wo tail is minimal.
  - PSUM: scores 4 banks (2 bufs x [128,1024]), PV accumulators 2 banks,
    shared proj/wo/transpose pool 2 banks = 8.
All HBM->SBUF loads use host-prearranged per-partition-contiguous layouts.
Compute dtype: bf16 matmul operands, fp32 PSUM accumulation, fp32 softmax.
"""

import numpy as np

B, S, D = 2, 2048, 2048
H, KVH, HD = 32, 8, 64
NCORES = 8
BS = B * S            # 4096
NHL = H // NCORES     # 4 q heads per core
MQ = NHL * HD         # 256 q columns per core
SBLK = 512
NSBLK = BS // SBLK    # 8
DC = D // 128         # 16 contraction chunks
NKCH = S // 128       # 16 key chunks per batch
NQS = S // SBLK       # 4 query blocks per batch


def build_graph(tc, out_ap, xT, wqEO, kvw, woS, c4, s4):
    """Build the per-core SPMD graph. All args are DRAM access patterns."""
    import concourse.mybir as mybir
    from concourse.masks import make_identity

    nc = tc.nc
    F32 = mybir.dt.float32
    BF16 = mybir.dt.bfloat16
    Alu = mybir.AluOpType
    Act = mybir.ActivationFunctionType
    TT = nc.vector.tensor_tensor
    CP = nc.vector.tensor_copy

    with tc.tile_pool(name="const", bufs=1) as constp, \
         tc.tile_pool(name="persist", bufs=1) as pers, \
         tc.tile_pool(name="dram", bufs=1, space="DRAM") as dramp:

        ident = constp.tile([128, 128], F32)
        make_identity(nc, ident[:])
        tri01f = constp.tile([128, 128], F32)
        nc.gpsimd.memset(tri01f[:], 1.0)
        nc.gpsimd.affine_select(
            out=tri01f[:], in_=tri01f[:], compare_op=Alu.is_ge, fill=0.0,
            base=0, channel_multiplier=-1, pattern=[[1, 128]])
        tri01 = constp.tile([128, 128], BF16)
        CP(tri01[:], tri01f[:])

        # weights/trig DMAs, ordered so the first projection chunk can start
        # ~2us in: wq g0, x(sb0), wq g1-3, kvw, trig
        wq_sb = constp.tile([128, DC, MQ], BF16)
        wqr = wqEO.rearrange("p (dc m) -> p dc m", dc=DC)
        nc.sync.dma_start(wq_sb[:, 0:4, :], wqr[:, 0:4, :])

        xTr = xT.rearrange("p (sb dc s) -> p sb dc s", sb=NSBLK, dc=DC)

        with tc.tile_pool(name="trig", bufs=1) as trigp, \
             tc.tile_pool(name="xtp", bufs=6) as xtp, \
             tc.tile_pool(name="ropep", bufs=2) as rp, \
             tc.tile_pool(name="ptp", bufs=2) as ptp, \
             tc.tile_pool(name="recp", bufs=2) as recp, \
             tc.tile_pool(name="wop", bufs=1) as wop, \
             tc.tile_pool(name="attsp", bufs=3) as attsp, \
             tc.tile_pool(name="outsp", bufs=2) as outsp:
            psA = psS = psO = psX = None  # PSUM pools, scoped in the driver

            def xt_dmas(sb):
                xts = []
                for g in range(4):
                    xt = xtp.tile([128, 4, SBLK], BF16, tag="xt", name=f"xt{g}")
                    nc.sync.dma_start(xt[:], xTr[:, sb, 4 * g:4 * g + 4, :])
                    xts.append(xt)
                return xts

            xts0 = xt_dmas(0)
            for g in range(1, 4):
                nc.sync.dma_start(wq_sb[:, 4 * g:4 * g + 4, :],
                                  wqr[:, 4 * g:4 * g + 4, :])
            # secondary weights/trig go via the gpsimd (SWDGE) queue so the
            # sync queue streams wq+x at full rate from t=0
            kvw_sb = constp.tile([128, DC, 128], BF16)
            nc.gpsimd.dma_start(kvw_sb[:], kvw.rearrange("p (dc m) -> p dc m", dc=DC))
            c4_sb = trigp.tile([128, S], F32)
            nc.gpsimd.dma_start(c4_sb[:], c4[:])
            s4_sb = trigp.tile([128, S], F32)
            nc.gpsimd.dma_start(s4_sb[:], s4[:])
            wo_sb = wop.tile([128, DC, MQ], BF16)
            nc.gpsimd.dma_start(wo_sb[:], woS.rearrange("p (dc m) -> p dc m", dc=DC))

            qt0 = pers.tile([128, BS], BF16)   # heads 0,1 (rows [0:64], [64:128])
            qt1 = pers.tile([128, BS], BF16)   # heads 2,3
            kT2 = pers.tile([128, BS], BF16)   # kT duplicated at base 0 and 64
            v1 = pers.tile([128, B * NKCH, 128], BF16)  # [v | ones] per chunk
            attnT0 = pers.tile([128, BS], BF16)
            attnT1 = pers.tile([128, BS], BF16)
            qts = [qt0, qt1]
            attnTs = [attnT0, attnT1]
            nc.gpsimd.memset(v1[:, :, 64:128], 1.0)

            def proj_sb(sb, xts, on_act, P):
                """Generator: projection+RoPE for one 512-col block of x^T.
                Yields ~1us-of-PE chunks. on_act: route copies to the scalar
                engine (idle outside attention) instead of DVE. P(tag) is the
                PSUM allocator (dedicated pool for b0, shared psX for b1)."""
                scol = slice(sb * SBLK, (sb + 1) * SBLK)
                pbeg = (sb % NQS) * SBLK
                qE_p = P("qE")
                for dc in range(DC):
                    nc.tensor.matmul(qE_p[:], wq_sb[:, dc, 0:128],
                                     xts[dc // 4][:, dc % 4],
                                     start=(dc == 0), stop=(dc == DC - 1))
                    if dc % 4 == 3:
                        yield
                qO_p = P("qO")
                for dc in range(DC):
                    nc.tensor.matmul(qO_p[:], wq_sb[:, dc, 128:256],
                                     xts[dc // 4][:, dc % 4],
                                     start=(dc == 0), stop=(dc == DC - 1))
                    if dc % 4 == 3:
                        yield
                c_ = c4_sb[:, pbeg:pbeg + SBLK]
                s_ = s4_sb[:, pbeg:pbeg + SBLK]
                # q RoPE: qE_p rows = 4 heads x 32 even comps, qO_p odd comps
                m1 = rp.tile([128, SBLK], F32, tag="m1")
                TT(m1[:], qE_p[:], c_, Alu.mult)
                m2 = rp.tile([128, SBLK], F32, tag="m2")
                TT(m2[:], qO_p[:], s_, Alu.mult)
                m3 = rp.tile([128, SBLK], F32, tag="m3")
                TT(m3[:], qO_p[:], c_, Alu.mult)
                m4 = rp.tile([128, SBLK], F32, tag="m4")
                TT(m4[:], qE_p[:], s_, Alu.mult)
                oE = rp.tile([128, SBLK], BF16, tag="oE")
                TT(oE[:], m1[:], m2[:], Alu.subtract)
                oO = rp.tile([128, SBLK], BF16, tag="oO")
                TT(oO[:], m3[:], m4[:], Alu.add)
                kv_p = P("kv")
                for dc in range(DC):
                    nc.tensor.matmul(kv_p[:], kvw_sb[:, dc, :],
                                     xts[dc // 4][:, dc % 4],
                                     start=(dc == 0), stop=(dc == DC - 1))
                    if dc % 4 == 3:
                        yield
                cp = nc.scalar.copy if on_act else CP
                # v first: evacuating vT promptly unblocks this PSUM slot for
                # the next block's chain (it's the slot-gating reader)
                vtw = rp.tile([64, SBLK], F32, tag="vtw")
                cp(vtw[:], kv_p[64:128, :])
                # k RoPE: kv_p rows [0:32]=kE, [32:64]=kO, [64:128]=vT
                a1 = rp.tile([32, SBLK], F32, tag="a1")
                TT(a1[:], kv_p[0:32, :], c_[0:32, :], Alu.mult)
                b1 = rp.tile([32, SBLK], F32, tag="b1")
                TT(b1[:], kv_p[32:64, :], s_[0:32, :], Alu.mult)
                a2 = rp.tile([32, SBLK], F32, tag="a2")
                TT(a2[:], kv_p[32:64, :], c_[0:32, :], Alu.mult)
                b2 = rp.tile([32, SBLK], F32, tag="b2")
                TT(b2[:], kv_p[0:32, :], s_[0:32, :], Alu.mult)
                TT(kT2[0:32, scol], a1[:], b1[:], Alu.subtract)
                TT(kT2[32:64, scol], a2[:], b2[:], Alu.add)
                yield
                tpx = P("tp")
                for q in range(4):
                    nc.tensor.transpose(tpx[:, 64 * q:64 * q + 64],
                                        vtw[:, q * 128:(q + 1) * 128],
                                        ident[0:64, 0:64])
                ch = sb * 4
                CP(v1[:, ch:ch + 4, 0:64],
                   tpx[:, 0:256].rearrange("p (c f) -> p c f", c=4))
                yield
                for h in range(NHL):
                    t, j = h // 2, h % 2
                    cp(qts[t][64 * j:64 * j + 32, scol], oE[32 * h:32 * h + 32, :])
                    cp(qts[t][64 * j + 32:64 * j + 64, scol], oO[32 * h:32 * h + 32, :])
                yield
                cp(kT2[64:96, scol], kT2[0:32, scol])
                cp(kT2[96:128, scol], kT2[32:64, scol])
                yield

            def proj_stream(sbs, on_act, P):
                """Chain proj generators with 1-block xt DMA lookahead."""
                pending = {sb: None for sb in sbs}
                pending[sbs[0]] = xts0 if sbs[0] == 0 else xt_dmas(sbs[0])
                for idx, sb in enumerate(sbs):
                    if idx + 1 < len(sbs):
                        pending[sbs[idx + 1]] = xt_dmas(sbs[idx + 1])
                    yield from proj_sb(sb, pending[sb], on_act, P)

            # ---- attention output segments: one per (b, qsub); the last
            # seg is gathered per head-pair so its t=0 half ships while t=1
            # attention still runs ----
            attnT_loc = [dramp.tile([MQ, SBLK], BF16, name=f"attnT_loc{i}")
                         for i in range(8)]
            attnT_all = [dramp.tile([D, SBLK], BF16, addr_space="Shared",
                                    name=f"attnT_all{i}") for i in range(8)]
            attnT_loc7 = [dramp.tile([128, SBLK], BF16, name=f"attnT_loc7{t}")
                          for t in range(2)]
            attnT_all7 = [dramp.tile([D // 2, SBLK], BF16, addr_space="Shared",
                                     name=f"attnT_all7{t}") for t in range(2)]

            def wo_dmas(seg):
                # gather-gated loads live on the gpsimd queue: on the sync
                # queue the scheduler hoists them ahead of xt loads and the
                # collective wait head-blocks the whole bulk-load stream
                attr = attnT_all[seg].rearrange("(dc p) s -> p dc s", p=128)
                atts = []
                for hf in range(2):
                    at = attsp.tile([128, DC // 2, SBLK], BF16, tag="att",
                                    name=f"att{hf}")
                    nc.gpsimd.dma_start(at[:], attr[:, 8 * hf:8 * hf + 8, :])
                    atts.append(at)
                return atts

            def wo_seg(seg, atts):
                """Generator: wo matmuls for one gathered 512-col segment."""
                b, qs = seg // NQS, seg % NQS
                for mc in range(2):
                    wp = psX.tile([128, SBLK], F32, tag="x", name="wp")
                    for dc in range(DC):
                        nc.tensor.matmul(
                            wp[:], wo_sb[:, dc, mc * 128:(mc + 1) * 128],
                            atts[dc // 8][:, dc % 8, :],
                            start=(dc == 0), stop=(dc == DC - 1))
                        if dc % 4 == 3:
                            yield
                    ot = outsp.tile([128, SBLK], F32, tag="ot")
                    CP(ot[:], wp[:])
                    nc.sync.dma_start(
                        out_ap[mc * 128:(mc + 1) * 128,
                               b * S + qs * SBLK:b * S + (qs + 1) * SBLK],
                        ot[:])
                    yield

            gathered_at = {}   # seg -> fill counter when its gather emitted
            fills = [0]        # shared fill-slot counter
            MARGIN = 12        # fill slots (~23us) for the gather to execute

            def wo_att7_load(t):
                attr = attnT_all7[t].rearrange("(c p) s -> p c s", p=128)
                at = attsp.tile([128, NCORES, SBLK], BF16, tag="att",
                                name=f"att7{t}")
                nc.gpsimd.dma_start(at[:], attr[:])
                return at

            def wo_seg7_half(t, at, wps):
                """One head-pair half of the final wo segment. t=0 opens the
                PSUM accumulation (runs while the t=1 gather is in flight);
                t=1 closes it and ships the output."""
                b, qs = 1, 3
                for mc in range(2):
                    if t == 0:
                        wps.append(psX.tile([128, SBLK], F32, tag="x",
                                            name="wp"))
                    wp = wps[mc]
                    for c in range(NCORES):
                        nc.tensor.matmul(
                            wp[:], wo_sb[:, 2 * c + t, mc * 128:(mc + 1) * 128],
                            at[:, c, :],
                            start=(t == 0 and c == 0),
                            stop=(t == 1 and c == NCORES - 1))
                    if t == 1:
                        ot = outsp.tile([128, SBLK], F32, tag="ot")
                        CP(ot[:], wp[:])
                        nc.sync.dma_start(
                            out_ap[mc * 128:(mc + 1) * 128,
                                   b * S + qs * SBLK:b * S + (qs + 1) * SBLK],
                            ot[:])

            def wo_stream(segs, preloaded=None):
                """wo segments with att-tile DMA prefetch one seg ahead,
                gated on the seg's gather having been emitted MARGIN fill
                slots ago (so the collective has likely completed)."""
                atts = dict(preloaded or {})

                def ready(s):
                    return s in gathered_at and \
                        fills[0] >= gathered_at[s] + MARGIN

                def ensure(s):
                    if s is not None and s not in atts and ready(s):
                        atts[s] = wo_dmas(s)

                def gen():
                    # let attention run ahead while the first att tiles land
                    for _ in range(10):
                        yield
                    for idx, seg in enumerate(segs):
                        nxt = segs[idx + 1] if idx + 1 < len(segs) else None
                        while not ready(seg):
                            yield
                        ensure(seg)
                        for _ in wo_seg(seg, atts.pop(seg)):
                            ensure(nxt)
                            yield
                return gen()

            def fill(gen):
                fills[0] += 1
                if gen is not None:
                    next(gen, None)

            rA, rB = slice(0, 64), slice(64, 128)

            def attn_batch(b, filler, fill_every, qs_order):
                """Attention for batch b, pulling filler chunks into the PE
                stream between iterations."""
                it = 0
                pend = []
                for qs in qs_order:
                    qcg = slice(b * S + qs * 512, b * S + (qs + 1) * 512)
                    for t in range(2):
                        oP = [psO.tile([128, SBLK], F32, tag=f"o{i}",
                                       name=f"o{i}") for i in range(2)]
                        # diagonal chunks first: the PV accumulation then
                        # starts and stops on full-width matmuls, with the
                        # narrowed diag writes in the middle
                        ks = list(range(4 * qs, 4 * qs + 4)) + \
                            list(range(0, 4 * qs))
                        for ki, k in enumerate(ks):
                            diag = (k // 4) == qs
                            es = 128 * (k % 4) if diag else 0
                            full_pv = qs == 0  # all-diag chain: pad with 0s
                            kc = slice(b * S + k * 128, b * S + k * 128 + 128)
                            qc = slice(b * S + qs * 512 + es,
                                       b * S + qs * 512 + 512)
                            sP = psS.tile([128, 1024], F32, tag="s", name="s")
                            nc.tensor.matmul(sP[:, es:512], kT2[rA, kc],
                                             qts[t][rA, qc],
                                             start=True, stop=True)
                            nc.tensor.matmul(sP[:, 512 + es:1024], kT2[rB, kc],
                                             qts[t][rB, qc],
                                             start=True, stop=True)
                            pP = ptp.tile([128, 1024], BF16, tag="p", name="p")
                            if es and full_pv:
                                nc.vector.memset(pP[:, 0:es], 0.0)
                                nc.vector.memset(pP[:, 512:512 + es], 0.0)
                            if es:
                                nc.scalar.activation(
                                    pP[:, es:512], sP[:, es:512],
                                    Act.Exp, scale=0.125)
                                nc.scalar.activation(
                                    pP[:, 512 + es:1024], sP[:, 512 + es:1024],
                                    Act.Exp, scale=0.125)
                            else:
                                nc.scalar.activation(
                                    pP[:], sP[:], Act.Exp, scale=0.125)
                            if diag:
                                # zero the causal triangle (key > q)
                                TT(pP[:, es:es + 128], pP[:, es:es + 128],
                                   tri01[:], Alu.mult)
                                TT(pP[:, 512 + es:512 + es + 128],
                                   pP[:, 512 + es:512 + es + 128],
                                   tri01[:], Alu.mult)
                            it += 1
                            if it % fill_every == 0:
                                fill(filler)
                            if pend:
                                pend.pop()()
                            def pv(pP=pP, k=k, ki=ki,
                                   es=(0 if full_pv else es)):
                                for i in range(2):
                                    nc.tensor.matmul(
                                        oP[i][:, es:512],
                                        v1[:, b * NKCH + k, :],
                                        pP[:, 512 * i + es:512 * i + 512],
                                        start=(ki == 0),
                                        stop=(ki == 4 * qs + 3),
                                        skip_group_check=(es != 0))
                            pend.append(pv)
                        if pend:
                            pend.pop()()
                        # normalize: oP rows [64:128] hold the denominator
                        for i, rows in enumerate((rA, rB)):
                            raw = recp.tile([128, SBLK], F32, tag=f"raw{i}",
                                            name=f"raw{i}")
                            CP(raw[:], oP[i][:])
                            den = recp.tile([64, SBLK], F32, tag=f"den{i}",
                                            name=f"den{i}")
                            CP(den[:], raw[64:128, :])
                            rec = recp.tile([64, SBLK], F32, tag=f"rec{i}",
                                            name=f"rec{i}")
                            nc.vector.reciprocal_approx_fast(rec[:], den[:])
                            TT(attnTs[t][rows, qcg], raw[0:64, :], rec[:],
                               Alu.mult)
                        if b * NQS + qs == 7:
                            nc.gpsimd.dma_start(attnT_loc7[t][:],
                                                attnTs[t][:, qcg])
                            nc.gpsimd.collective_compute(
                                "AllGather", mybir.AluOpType.bypass,
                                replica_groups=[list(range(NCORES))],
                                ins=[attnT_loc7[t].opt()],
                                outs=[attnT_all7[t].opt()])
                            gathered_at[(7, t)] = fills[0]
                        fill(filler)
                    seg = b * NQS + qs
                    if seg < 7:
                        nc.gpsimd.dma_start(attnT_loc[seg][0:128, :],
                                            attnTs[0][:, qcg])
                        nc.gpsimd.dma_start(attnT_loc[seg][128:256, :],
                                            attnTs[1][:, qcg])
                        nc.gpsimd.collective_compute(
                            "AllGather", mybir.AluOpType.bypass,
                            replica_groups=[list(range(NCORES))],
                            ins=[attnT_loc[seg].opt()],
                            outs=[attnT_all[seg].opt()])
                        gathered_at[seg] = fills[0]

            # ---- the pipelined program ----
            def PA(tag):
                return psA.tile([128, SBLK], F32, tag=tag, name=tag)

            def PX(tag):
                return psX.tile([128, SBLK], F32, tag="x", name=tag)

            with tc.tile_pool(name="psA", bufs=2, space="PSUM") as psA:
                projA = proj_stream([0, 1, 2, 3], True, PA)
                for _ in projA:
                    pass
            with tc.tile_pool(name="psS", bufs=2, space="PSUM") as psS, \
                 tc.tile_pool(name="psO", bufs=1, space="PSUM") as psO, \
                 tc.tile_pool(name="psX", bufs=2, space="PSUM") as psX:
                projB = proj_stream([4, 5, 6, 7], False, PX)

                # b0's attention is PE-bound: fill it with b1 proj only.
                # b1's attention is gather-paced (has slack): all wo goes
                # there. Seg order matches gather production (b1 runs qs
                # [2,0,1,3] -> segs 6,4,5 then the split 7).
                attn_batch(0, projB, 1, [0, 1, 2, 3])
                for _ in projB:    # ensure batch-1 proj is fully emitted
                    pass
                pre = {s: wo_dmas(s) for s in (0, 1)}
                woA = wo_stream([0, 1, 2, 3, 6, 4, 5], pre)
                # qs2 first: its gather (the would-be tail straggler) fires
                # early; qs3's split gather halves are then the only tail work
                attn_batch(1, woA, 1, [2, 0, 1, 3])
                fills[0] = 1 << 30   # drain: no more fill slots, no gating
                for _ in woA:
                    pass
                # final segment: the t=0 half opens the PSUM accumulation as
                # soon as its gather lands, overlapping the t=1 gather wait
                wps = []
                wo_seg7_half(0, wo_att7_load(0), wps)
                wo_seg7_half(1, wo_att7_load(1), wps)


def prep_inputs(x, cos, sin, wq, wk, wv, wo):
    """Host-side layout prep. Returns per-core input dicts (bf16/f32).

    All SBUF-bound tensors are prearranged so that each of the 128 SBUF
    partitions reads one contiguous DRAM chunk (fat DMA descriptors).
    """
    import ml_dtypes
    bf16 = ml_dtypes.bfloat16
    x = np.asarray(x, np.float32)
    cos = np.asarray(cos, np.float32)
    sin = np.asarray(sin, np.float32)
    wq = np.asarray(wq, np.float32)
    wk = np.asarray(wk, np.float32)
    wv = np.asarray(wv, np.float32)
    wo = np.asarray(wo, np.float32)

    xT = np.ascontiguousarray(x.transpose(2, 0, 1).reshape(D, BS)).astype(bf16)
    # [dc*128+p, sb*512+s] -> [p, sb, dc, s] flattened per partition
    xTc = np.ascontiguousarray(
        xT.reshape(DC, 128, NSBLK, SBLK).transpose(1, 2, 0, 3).reshape(128, -1))
    c4 = np.ascontiguousarray(np.tile(cos.T, (4, 1)))          # [128, S] f32
    s4 = np.ascontiguousarray(np.tile(sin.T, (4, 1)))
    eperm = np.array([64 * h + 2 * j for h in range(NHL) for j in range(32)])
    operm = eperm + 1

    def pmajor(w):  # [D, M] -> [128, DC*M] with [p, dc, m] contiguous
        m = w.shape[1]
        return np.ascontiguousarray(
            w.reshape(DC, 128, m).transpose(1, 0, 2).reshape(128, -1)).astype(bf16)

    in_maps = []
    for c in range(NCORES):
        wq_sh = wq[:, MQ * c:MQ * c + MQ]
        wqEO = np.concatenate([wq_sh[:, eperm], wq_sh[:, operm]], axis=1)
        kc = wk[:, HD * c:HD * c + HD]
        vc = wv[:, HD * c:HD * c + HD]
        kvw = np.concatenate([kc[:, 0::2], kc[:, 1::2], vc], axis=1)
        woS = wo[:, MQ * c:MQ * c + MQ]
        in_maps.append({
            "xT": xTc,
            "wqEO": pmajor(wqEO),
            "kvw": pmajor(kvw),
            "woS": pmajor(woS),
            "c4": c4,
            "s4": s4,
        })
    return in_maps


def assemble_output(core_outs):
    """core_outs: list of 8 [256, BS] f32 arrays -> [B, S, D] f32."""
    outT = np.concatenate(core_outs, axis=0)           # [D, BS]
    return np.ascontiguousarray(
        outT.reshape(D, B, S).transpose(1, 2, 0)).astype(np.float32)


_CACHE = {}


def _get_compiled():
    if "nc" in _CACHE:
        return _CACHE["nc"]
    import concourse.mybir as mybir
    import concourse.tile as tile
    from concourse import bacc

    nc = bacc.Bacc("TRN2", target_bir_lowering=False, debug=False,
                   num_devices=NCORES)
    F32 = mybir.dt.float32
    BF16 = mybir.dt.bfloat16
    xT_d = nc.dram_tensor("xT", [128, NSBLK * DC * SBLK], BF16, kind="ExternalInput")
    wq_d = nc.dram_tensor("wqEO", [128, DC * MQ], BF16, kind="ExternalInput")
    kvw_d = nc.dram_tensor("kvw", [128, DC * 128], BF16, kind="ExternalInput")
    wo_d = nc.dram_tensor("woS", [128, DC * MQ], BF16, kind="ExternalInput")
    c4_d = nc.dram_tensor("c4", [128, S], F32, kind="ExternalInput")
    s4_d = nc.dram_tensor("s4", [128, S], F32, kind="ExternalInput")
    out_d = nc.dram_tensor("out", [MQ, BS], F32, kind="ExternalOutput")
    with tile.TileContext(nc) as tc:
        build_graph(tc, out_d.ap(), xT_d.ap(), wq_d.ap(), kvw_d.ap(),
                    wo_d.ap(), c4_d.ap(), s4_d.ap())
    nc.compile()
    _CACHE["nc"] = nc
    return nc


def kernel(x, cos, sin, wq, wk, wv, wo):
    from concourse.bass_utils import run_bass_kernel_spmd
    nc = _get_compiled()
    in_maps = prep_inputs(x, cos, sin, wq, wk, wv, wo)
    res = run_bass_kernel_spmd(nc, in_maps, core_ids=list(range(NCORES)))
    _CACHE["last_results"] = res
    return assemble_output([res.results[c]["out"] for c in range(NCORES)])



# revision 15
# speedup vs baseline: 1.0421x; 1.0135x over previous
"""GQA attention (B=2, S=2048, D=2048, 32 Q heads / 8 KV heads, HD=64, RoPE,
causal) on 8 TRN2 NeuronCores.

Sharding: tensor-parallel over heads. Core c owns q heads [4c, 4c+4) and kv
head c (GQA groups align exactly with 8 cores); both batches replicated.

Fully software-pipelined single stream per core:
  - batch-0 projections (qkv + RoPE, transposed layout) run PE-dense;
  - batch-0 attention (s^T layout, merged [128,1024] exp on ACT, ones-row
    softmax denominators) is exp-latency-bound on the scalar engine, so
    batch-1 projection matmul chunks are interleaved into the PE stream to
    fill the gaps; batch-1 attention interleaves the batch-0 wo matmuls the
    same way.
  - attention output ships per 512-query block: 8 AllGather segments
    (bf16, 0.25MB/rank) fired as soon as each block is normalized, with the
    qsub blocks processed high-to-low so the final (smallest) block's
    gather+wo tail is minimal.
  - PSUM: scores 4 banks (2 bufs x [128,1024]), PV accumulators 2 banks,
    shared proj/wo/transpose pool 2 banks = 8.
All HBM->SBUF loads use host-prearranged per-partition-contiguous layouts.
Compute dtype: bf16 matmul operands, fp32 PSUM accumulation, fp32 softmax.
"""

import numpy as np

B, S, D = 2, 2048, 2048
H, KVH, HD = 32, 8, 64
NCORES = 8
BS = B * S            # 4096
NHL = H // NCORES     # 4 q heads per core
MQ = NHL * HD         # 256 q columns per core
SBLK = 512
NSBLK = BS // SBLK    # 8
DC = D // 128         # 16 contraction chunks
NKCH = S // 128       # 16 key chunks per batch
NQS = S // SBLK       # 4 query blocks per batch


def build_graph(tc, out_ap, xT, wqEO, kvw, woS, c4, s4):
    """Build the per-core SPMD graph. All args are DRAM access patterns."""
    import concourse.mybir as mybir
    from concourse.masks import make_identity

    nc = tc.nc
    F32 = mybir.dt.float32
    BF16 = mybir.dt.bfloat16
    Alu = mybir.AluOpType
    Act = mybir.ActivationFunctionType
    TT = nc.vector.tensor_tensor
    CP = nc.vector.tensor_copy

    with tc.tile_pool(name="const", bufs=1) as constp, \
         tc.tile_pool(name="persist", bufs=1) as pers, \
         tc.tile_pool(name="dram", bufs=1, space="DRAM") as dramp:

        ident = constp.tile([128, 128], F32)
        make_identity(nc, ident[:])
        tri01f = constp.tile([128, 128], F32)
        nc.gpsimd.memset(tri01f[:], 1.0)
        nc.gpsimd.affine_select(
            out=tri01f[:], in_=tri01f[:], compare_op=Alu.is_ge, fill=0.0,
            base=0, channel_multiplier=-1, pattern=[[1, 128]])
        tri01 = constp.tile([128, 128], BF16)
        CP(tri01[:], tri01f[:])

        # weights/trig DMAs, ordered so the first projection chunk can start
        # ~2us in: wq g0, x(sb0), wq g1-3, kvw, trig
        wq_sb = constp.tile([128, DC, MQ], BF16)
        wqr = wqEO.rearrange("p (dc m) -> p dc m", dc=DC)
        nc.sync.dma_start(wq_sb[:, 0:4, :], wqr[:, 0:4, :])

        xTr = xT.rearrange("p (sb dc s) -> p sb dc s", sb=NSBLK, dc=DC)

        with tc.tile_pool(name="trig", bufs=1) as trigp, \
             tc.tile_pool(name="xtp", bufs=6) as xtp, \
             tc.tile_pool(name="ropep", bufs=2) as rp, \
             tc.tile_pool(name="ptp", bufs=2) as ptp, \
             tc.tile_pool(name="recp", bufs=2) as recp, \
             tc.tile_pool(name="wop", bufs=1) as wop, \
             tc.tile_pool(name="attsp", bufs=3) as attsp, \
             tc.tile_pool(name="outsp", bufs=2) as outsp:
            psA = psS = psO = psX = None  # PSUM pools, scoped in the driver

            def xt_dmas(sb):
                xts = []
                for g in range(4):
                    xt = xtp.tile([128, 4, SBLK], BF16, tag="xt", name=f"xt{g}")
                    nc.sync.dma_start(xt[:], xTr[:, sb, 4 * g:4 * g + 4, :])
                    xts.append(xt)
                return xts

            xts0 = xt_dmas(0)
            for g in range(1, 4):
                nc.sync.dma_start(wq_sb[:, 4 * g:4 * g + 4, :],
                                  wqr[:, 4 * g:4 * g + 4, :])
            # secondary weights/trig go via the gpsimd (SWDGE) queue so the
            # sync queue streams wq+x at full rate from t=0
            kvw_sb = constp.tile([128, DC, 128], BF16)
            nc.gpsimd.dma_start(kvw_sb[:], kvw.rearrange("p (dc m) -> p dc m", dc=DC))
            c4_sb = trigp.tile([128, S], F32)
            nc.gpsimd.dma_start(c4_sb[:], c4[:])
            s4_sb = trigp.tile([128, S], F32)
            nc.gpsimd.dma_start(s4_sb[:], s4[:])
            wo_sb = wop.tile([128, DC, MQ], BF16)
            nc.gpsimd.dma_start(wo_sb[:], woS.rearrange("p (dc m) -> p dc m", dc=DC))

            qt0 = pers.tile([128, BS], BF16)   # heads 0,1 (rows [0:64], [64:128])
            qt1 = pers.tile([128, BS], BF16)   # heads 2,3
            kT2 = pers.tile([128, BS], BF16)   # kT duplicated at base 0 and 64
            v1 = pers.tile([128, B * NKCH, 128], BF16)  # [v | ones] per chunk
            attnT0 = pers.tile([128, BS], BF16)
            attnT1 = pers.tile([128, BS], BF16)
            qts = [qt0, qt1]
            attnTs = [attnT0, attnT1]
            nc.gpsimd.memset(v1[:, :, 64:128], 1.0)

            def proj_sb(sb, xts, on_act, P):
                """Generator: projection+RoPE for one 512-col block of x^T.
                Yields ~1us-of-PE chunks. on_act: route copies to the scalar
                engine (idle outside attention) instead of DVE. P(tag) is the
                PSUM allocator (dedicated pool for b0, shared psX for b1)."""
                scol = slice(sb * SBLK, (sb + 1) * SBLK)
                pbeg = (sb % NQS) * SBLK
                qE_p = P("qE")
                for dc in range(DC):
                    nc.tensor.matmul(qE_p[:], wq_sb[:, dc, 0:128],
                                     xts[dc // 4][:, dc % 4],
                                     start=(dc == 0), stop=(dc == DC - 1))
                    if dc % 4 == 3:
                        yield
                qO_p = P("qO")
                for dc in range(DC):
                    nc.tensor.matmul(qO_p[:], wq_sb[:, dc, 128:256],
                                     xts[dc // 4][:, dc % 4],
                                     start=(dc == 0), stop=(dc == DC - 1))
                    if dc % 4 == 3:
                        yield
                c_ = c4_sb[:, pbeg:pbeg + SBLK]
                s_ = s4_sb[:, pbeg:pbeg + SBLK]
                # q RoPE: qE_p rows = 4 heads x 32 even comps, qO_p odd comps
                m1 = rp.tile([128, SBLK], F32, tag="m1")
                TT(m1[:], qE_p[:], c_, Alu.mult)
                m2 = rp.tile([128, SBLK], F32, tag="m2")
                TT(m2[:], qO_p[:], s_, Alu.mult)
                m3 = rp.tile([128, SBLK], F32, tag="m3")
                TT(m3[:], qO_p[:], c_, Alu.mult)
                m4 = rp.tile([128, SBLK], F32, tag="m4")
                TT(m4[:], qE_p[:], s_, Alu.mult)
                oE = rp.tile([128, SBLK], BF16, tag="oE")
                TT(oE[:], m1[:], m2[:], Alu.subtract)
                oO = rp.tile([128, SBLK], BF16, tag="oO")
                TT(oO[:], m3[:], m4[:], Alu.add)
                kv_p = P("kv")
                for dc in range(DC):
                    nc.tensor.matmul(kv_p[:], kvw_sb[:, dc, :],
                                     xts[dc // 4][:, dc % 4],
                                     start=(dc == 0), stop=(dc == DC - 1))
                    if dc % 4 == 3:
                        yield
                cp = nc.scalar.copy if on_act else CP
                # v first: evacuating vT promptly unblocks this PSUM slot for
                # the next block's chain (it's the slot-gating reader)
                vtw = rp.tile([64, SBLK], F32, tag="vtw")
                cp(vtw[:], kv_p[64:128, :])
                # k RoPE: kv_p rows [0:32]=kE, [32:64]=kO, [64:128]=vT
                a1 = rp.tile([32, SBLK], F32, tag="a1")
                TT(a1[:], kv_p[0:32, :], c_[0:32, :], Alu.mult)
                b1 = rp.tile([32, SBLK], F32, tag="b1")
                TT(b1[:], kv_p[32:64, :], s_[0:32, :], Alu.mult)
                a2 = rp.tile([32, SBLK], F32, tag="a2")
                TT(a2[:], kv_p[32:64, :], c_[0:32, :], Alu.mult)
                b2 = rp.tile([32, SBLK], F32, tag="b2")
                TT(b2[:], kv_p[0:32, :], s_[0:32, :], Alu.mult)
                TT(kT2[0:32, scol], a1[:], b1[:], Alu.subtract)
                TT(kT2[32:64, scol], a2[:], b2[:], Alu.add)
                yield
                tpx = P("tp")
                for q in range(4):
                    nc.tensor.transpose(tpx[:, 64 * q:64 * q + 64],
                                        vtw[:, q * 128:(q + 1) * 128],
                                        ident[0:64, 0:64])
                ch = sb * 4
                CP(v1[:, ch:ch + 4, 0:64],
                   tpx[:, 0:256].rearrange("p (c f) -> p c f", c=4))
                yield
                for h in range(NHL):
                    t, j = h // 2, h % 2
                    cp(qts[t][64 * j:64 * j + 32, scol], oE[32 * h:32 * h + 32, :])
                    cp(qts[t][64 * j + 32:64 * j + 64, scol], oO[32 * h:32 * h + 32, :])
                yield
                cp(kT2[64:96, scol], kT2[0:32, scol])
                cp(kT2[96:128, scol], kT2[32:64, scol])
                yield

            def proj_stream(sbs, on_act, P):
                """Chain proj generators with 1-block xt DMA lookahead."""
                pending = {sb: None for sb in sbs}
                pending[sbs[0]] = xts0 if sbs[0] == 0 else xt_dmas(sbs[0])
                for idx, sb in enumerate(sbs):
                    if idx + 1 < len(sbs):
                        pending[sbs[idx + 1]] = xt_dmas(sbs[idx + 1])
                    yield from proj_sb(sb, pending[sb], on_act, P)

            # ---- attention output segments: one per (b, qsub); the last
            # seg is gathered per head-pair so its t=0 half ships while t=1
            # attention still runs ----
            attnT_loc = [dramp.tile([MQ, SBLK], BF16, name=f"attnT_loc{i}")
                         for i in range(8)]
            attnT_all = [dramp.tile([D, SBLK], BF16, addr_space="Shared",
                                    name=f"attnT_all{i}") for i in range(8)]
            attnT_loc7 = [dramp.tile([128, SBLK], BF16, name=f"attnT_loc7{t}")
                          for t in range(2)]
            attnT_all7 = [dramp.tile([D // 2, SBLK], BF16, addr_space="Shared",
                                     name=f"attnT_all7{t}") for t in range(2)]

            def wo_dmas(seg):
                # gather-gated loads live on the gpsimd queue: on the sync
                # queue the scheduler hoists them ahead of xt loads and the
                # collective wait head-blocks the whole bulk-load stream
                attr = attnT_all[seg].rearrange("(dc p) s -> p dc s", p=128)
                atts = []
                for hf in range(2):
                    at = attsp.tile([128, DC // 2, SBLK], BF16, tag="att",
                                    name=f"att{hf}")
                    nc.gpsimd.dma_start(at[:], attr[:, 8 * hf:8 * hf + 8, :])
                    atts.append(at)
                return atts

            def wo_seg(seg, atts):
                """Generator: wo matmuls for one gathered 512-col segment."""
                b, qs = seg // NQS, seg % NQS
                for mc in range(2):
                    wp = psX.tile([128, SBLK], F32, tag="x", name="wp")
                    for dc in range(DC):
                        nc.tensor.matmul(
                            wp[:], wo_sb[:, dc, mc * 128:(mc + 1) * 128],
                            atts[dc // 8][:, dc % 8, :],
                            start=(dc == 0), stop=(dc == DC - 1))
                        if dc % 4 == 3:
                            yield
                    ot = outsp.tile([128, SBLK], F32, tag="ot")
                    CP(ot[:], wp[:])
                    nc.sync.dma_start(
                        out_ap[mc * 128:(mc + 1) * 128,
                               b * S + qs * SBLK:b * S + (qs + 1) * SBLK],
                        ot[:])
                    yield

            gathered_at = {}   # seg -> fill counter when its gather emitted
            fills = [0]        # shared fill-slot counter
            MARGIN = 12        # fill slots (~23us) for the gather to execute

            def wo_att7_load(t):
                attr = attnT_all7[t].rearrange("(c p) s -> p c s", p=128)
                at = attsp.tile([128, NCORES, SBLK], BF16, tag="att",
                                name=f"att7{t}")
                nc.gpsimd.dma_start(at[:], attr[:])
                return at

            def wo_seg7_half(t, at, wps):
                """One head-pair half of the final wo segment. t=0 opens the
                PSUM accumulation (runs while the t=1 gather is in flight);
                t=1 closes it and ships the output."""
                b, qs = 1, 3
                for mc in range(2):
                    if t == 0:
                        wps.append(psX.tile([128, SBLK], F32, tag="x",
                                            name="wp"))
                    wp = wps[mc]
                    for c in range(NCORES):
                        nc.tensor.matmul(
                            wp[:], wo_sb[:, 2 * c + t, mc * 128:(mc + 1) * 128],
                            at[:, c, :],
                            start=(t == 0 and c == 0),
                            stop=(t == 1 and c == NCORES - 1))
                    if t == 1:
                        ot = outsp.tile([128, SBLK], F32, tag="ot")
                        CP(ot[:], wp[:])
                        nc.sync.dma_start(
                            out_ap[mc * 128:(mc + 1) * 128,
                                   b * S + qs * SBLK:b * S + (qs + 1) * SBLK],
                            ot[:])

            def wo_stream(segs, preloaded=None):
                """wo segments with att-tile DMA prefetch one seg ahead,
                gated on the seg's gather having been emitted MARGIN fill
                slots ago (so the collective has likely completed)."""
                atts = dict(preloaded or {})

                def ready(s):
                    return s in gathered_at and \
                        fills[0] >= gathered_at[s] + MARGIN

                def ensure(s):
                    if s is not None and s not in atts and ready(s):
                        atts[s] = wo_dmas(s)

                def gen():
                    # let attention run ahead while the first att tiles land
                    for _ in range(10):
                        yield
                    for idx, seg in enumerate(segs):
                        nxt = segs[idx + 1] if idx + 1 < len(segs) else None
                        while not ready(seg):
                            yield
                        ensure(seg)
                        for _ in wo_seg(seg, atts.pop(seg)):
                            ensure(nxt)
                            yield
                return gen()

            def fill(gen):
                fills[0] += 1
                if gen is not None:
                    next(gen, None)

            rA, rB = slice(0, 64), slice(64, 128)

            def attn_batch(b, filler, fill_every, qs_order):
                """Attention for batch b, pulling filler chunks into the PE
                stream between iterations."""
                it = 0
                pend = []
                for qs in qs_order:
                    qcg = slice(b * S + qs * 512, b * S + (qs + 1) * 512)
                    for t in range(2):
                        oP = [psO.tile([128, SBLK], F32, tag=f"o{i}",
                                       name=f"o{i}") for i in range(2)]
                        # diagonal chunks first: the PV accumulation then
                        # starts and stops on full-width matmuls, with the
                        # narrowed diag writes in the middle
                        ks = list(range(4 * qs, 4 * qs + 4)) + \
                            list(range(0, 4 * qs))
                        for ki, k in enumerate(ks):
                            diag = (k // 4) == qs
                            es = 128 * (k % 4) if diag else 0
                            full_pv = qs == 0  # all-diag chain: pad with 0s
                            kc = slice(b * S + k * 128, b * S + k * 128 + 128)
                            qc = slice(b * S + qs * 512 + es,
                                       b * S + qs * 512 + 512)
                            sP = psS.tile([128, 1024], F32, tag="s", name="s")
                            nc.tensor.matmul(sP[:, es:512], kT2[rA, kc],
                                             qts[t][rA, qc],
                                             start=True, stop=True)
                            nc.tensor.matmul(sP[:, 512 + es:1024], kT2[rB, kc],
                                             qts[t][rB, qc],
                                             start=True, stop=True)
                            pP = ptp.tile([128, 1024], BF16, tag="p", name="p")
                            if es and full_pv:
                                nc.vector.memset(pP[:, 0:es], 0.0)
                                nc.vector.memset(pP[:, 512:512 + es], 0.0)
                            if es:
                                nc.scalar.activation(
                                    pP[:, es:512], sP[:, es:512],
                                    Act.Exp, scale=0.125)
                                nc.scalar.activation(
                                    pP[:, 512 + es:1024], sP[:, 512 + es:1024],
                                    Act.Exp, scale=0.125)
                            else:
                                nc.scalar.activation(
                                    pP[:], sP[:], Act.Exp, scale=0.125)
                            if diag:
                                # zero the causal triangle (key > q)
                                TT(pP[:, es:es + 128], pP[:, es:es + 128],
                                   tri01[:], Alu.mult)
                                TT(pP[:, 512 + es:512 + es + 128],
                                   pP[:, 512 + es:512 + es + 128],
                                   tri01[:], Alu.mult)
                            it += 1
                            if it % fill_every == 0:
                                fill(filler)
                            if pend:
                                pend.pop()()
                            def pv(pP=pP, k=k, ki=ki,
                                   es=(0 if full_pv else es)):
                                for i in range(2):
                                    nc.tensor.matmul(
                                        oP[i][:, es:512],
                                        v1[:, b * NKCH + k, :],
                                        pP[:, 512 * i + es:512 * i + 512],
                                        start=(ki == 0),
                                        stop=(ki == 4 * qs + 3),
                                        skip_group_check=(es != 0))
                            pend.append(pv)
                        if pend:
                            pend.pop()()
                        # normalize: oP rows [64:128] hold the denominator
                        for i, rows in enumerate((rA, rB)):
                            raw = recp.tile([128, SBLK], F32, tag=f"raw{i}",
                                            name=f"raw{i}")
                            CP(raw[:], oP[i][:])
                            den = recp.tile([64, SBLK], F32, tag=f"den{i}",
                                            name=f"den{i}")
                            CP(den[:], raw[64:128, :])
                            rec = recp.tile([64, SBLK], F32, tag=f"rec{i}",
                                            name=f"rec{i}")
                            nc.vector.reciprocal_approx_fast(rec[:], den[:])
                            TT(attnTs[t][rows, qcg], raw[0:64, :], rec[:],
                               Alu.mult)
                        if b * NQS + qs == 7:
                            nc.sync.dma_start(attnT_loc7[t][:],
                                              attnTs[t][:, qcg])
                            nc.gpsimd.collective_compute(
                                "AllGather", mybir.AluOpType.bypass,
                                replica_groups=[list(range(NCORES))],
                                ins=[attnT_loc7[t].opt()],
                                outs=[attnT_all7[t].opt()])
                            gathered_at[(7, t)] = fills[0]
                        fill(filler)
                    seg = b * NQS + qs
                    if seg < 7:
                        nc.sync.dma_start(attnT_loc[seg][0:128, :],
                                          attnTs[0][:, qcg])
                        nc.sync.dma_start(attnT_loc[seg][128:256, :],
                                          attnTs[1][:, qcg])
                        nc.gpsimd.collective_compute(
                            "AllGather", mybir.AluOpType.bypass,
                            replica_groups=[list(range(NCORES))],
                            ins=[attnT_loc[seg].opt()],
                            outs=[attnT_all[seg].opt()])
                        gathered_at[seg] = fills[0]

            # ---- the pipelined program ----
            def PA(tag):
                return psA.tile([128, SBLK], F32, tag=tag, name=tag)

            def PX(tag):
                return psX.tile([128, SBLK], F32, tag="x", name=tag)

            with tc.tile_pool(name="psA", bufs=2, space="PSUM") as psA:
                projA = proj_stream([0, 1, 2, 3], True, PA)
                for _ in projA:
                    pass
            with tc.tile_pool(name="psS", bufs=2, space="PSUM") as psS, \
                 tc.tile_pool(name="psO", bufs=1, space="PSUM") as psO, \
                 tc.tile_pool(name="psX", bufs=2, space="PSUM") as psX:
                projB = proj_stream([4, 5, 6, 7], False, PX)

                # b0's attention is PE-bound: fill it with b1 proj only.
                # b1's attention is gather-paced (has slack): all wo goes
                # there. Seg order matches gather production (b1 runs qs
                # [2,0,1,3] -> segs 6,4,5 then the split 7).
                attn_batch(0, projB, 1, [0, 1, 2, 3])
                for _ in projB:    # ensure batch-1 proj is fully emitted
                    pass
                pre = {s: wo_dmas(s) for s in (0, 1)}
                woA = wo_stream([0, 1, 2, 3, 6, 4, 5], pre)
                # qs2 first: its gather (the would-be tail straggler) fires
                # early; qs3's split gather halves are then the only tail work
                attn_batch(1, woA, 1, [2, 0, 1, 3])
                fills[0] = 1 << 30   # drain: no more fill slots, no gating
                for _ in woA:
                    pass
                # final segment: the t=0 half opens the PSUM accumulation as
                # soon as its gather lands, overlapping the t=1 gather wait
                wps = []
                wo_seg7_half(0, wo_att7_load(0), wps)
                wo_seg7_half(1, wo_att7_load(1), wps)


def prep_inputs(x, cos, sin, wq, wk, wv, wo):
    """Host-side layout prep. Returns per-core input dicts (bf16/f32).

    All SBUF-bound tensors are prearranged so that each of the 128 SBUF
    partitions reads one contiguous DRAM chunk (fat DMA descriptors).
    """
    import ml_dtypes
    bf16 = ml_dtypes.bfloat16
    x = np.asarray(x, np.float32)
    cos = np.asarray(cos, np.float32)
    sin = np.asarray(sin, np.float32)
    wq = np.asarray(wq, np.float32)
    wk = np.asarray(wk, np.float32)
    wv = np.asarray(wv, np.float32)
    wo = np.asarray(wo, np.float32)

    xT = np.ascontiguousarray(x.transpose(2, 0, 1).reshape(D, BS)).astype(bf16)
    # [dc*128+p, sb*512+s] -> [p, sb, dc, s] flattened per partition
    xTc = np.ascontiguousarray(
        xT.reshape(DC, 128, NSBLK, SBLK).transpose(1, 2, 0, 3).reshape(128, -1))
    c4 = np.ascontiguousarray(np.tile(cos.T, (4, 1)))          # [128, S] f32
    s4 = np.ascontiguousarray(np.tile(sin.T, (4, 1)))
    eperm = np.array([64 * h + 2 * j for h in range(NHL) for j in range(32)])
    operm = eperm + 1

    def pmajor(w):  # [D, M] -> [128, DC*M] with [p, dc, m] contiguous
        m = w.shape[1]
        return np.ascontiguousarray(
            w.reshape(DC, 128, m).transpose(1, 0, 2).reshape(128, -1)).astype(bf16)

    in_maps = []
    for c in range(NCORES):
        wq_sh = wq[:, MQ * c:MQ * c + MQ]
        wqEO = np.concatenate([wq_sh[:, eperm], wq_sh[:, operm]], axis=1)
        kc = wk[:, HD * c:HD * c + HD]
        vc = wv[:, HD * c:HD * c + HD]
        kvw = np.concatenate([kc[:, 0::2], kc[:, 1::2], vc], axis=1)
        woS = wo[:, MQ * c:MQ * c + MQ]
        in_maps.append({
            "xT": xTc,
            "wqEO": pmajor(wqEO),
            "kvw": pmajor(kvw),
            "woS": pmajor(woS),
            "c4": c4,
            "s4": s4,
        })
    return in_maps


def assemble_output(core_outs):
    """core_outs: list of 8 [256, BS] f32 arrays -> [B, S, D] f32."""
    outT = np.concatenate(core_outs, axis=0)           # [D, BS]
    return np.ascontiguousarray(
        outT.reshape(D, B, S).transpose(1, 2, 0)).astype(np.float32)


_CACHE = {}


def _get_compiled():
    if "nc" in _CACHE:
        return _CACHE["nc"]
    import concourse.mybir as mybir
    import concourse.tile as tile
    from concourse import bacc

    nc = bacc.Bacc("TRN2", target_bir_lowering=False, debug=False,
                   num_devices=NCORES)
    F32 = mybir.dt.float32
    BF16 = mybir.dt.bfloat16
    xT_d = nc.dram_tensor("xT", [128, NSBLK * DC * SBLK], BF16, kind="ExternalInput")
    wq_d = nc.dram_tensor("wqEO", [128, DC * MQ], BF16, kind="ExternalInput")
    kvw_d = nc.dram_tensor("kvw", [128, DC * 128], BF16, kind="ExternalInput")
    wo_d = nc.dram_tensor("woS", [128, DC * MQ], BF16, kind="ExternalInput")
    c4_d = nc.dram_tensor("c4", [128, S], F32, kind="ExternalInput")
    s4_d = nc.dram_tensor("s4", [128, S], F32, kind="ExternalInput")
    out_d = nc.dram_tensor("out", [MQ, BS], F32, kind="ExternalOutput")
    with tile.TileContext(nc) as tc:
        build_graph(tc, out_d.ap(), xT_d.ap(), wq_d.ap(), kvw_d.ap(),
                    wo_d.ap(), c4_d.ap(), s4_d.ap())
    nc.compile()
    _CACHE["nc"] = nc
    return nc


def kernel(x, cos, sin, wq, wk, wv, wo):
    from concourse.bass_utils import run_bass_kernel_spmd
    nc = _get_compiled()
    in_maps = prep_inputs(x, cos, sin, wq, wk, wv, wo)
    res = run_bass_kernel_spmd(nc, in_maps, core_ids=list(range(NCORES)))
    _CACHE["last_results"] = res
    return assemble_output([res.results[c]["out"] for c in range(NCORES)])



# revision 17
# speedup vs baseline: 1.1561x; 1.1094x over previous
"""GQA attention (B=2, S=2048, D=2048, 32 Q heads / 8 KV heads, HD=64, RoPE,
causal) on 8 TRN2 NeuronCores.

Sharding: tensor-parallel over heads. Core c owns q heads [4c, 4c+4) and kv
head c (GQA groups align exactly with 8 cores); both batches replicated.

Fully software-pipelined single stream per core:
  - batch-0 projections (qkv + RoPE, transposed layout) run PE-dense;
  - batch-0 attention (s^T layout, merged [128,1024] exp on ACT, ones-row
    softmax denominators) is exp-latency-bound on the scalar engine, so
    batch-1 projection matmul chunks are interleaved into the PE stream to
    fill the gaps; batch-1 attention interleaves the batch-0 wo matmuls the
    same way.
  - attention output ships per 512-query block: 8 AllGather segments
    (bf16, 0.25MB/rank) fired as soon as each block is normalized, with the
    qsub blocks processed high-to-low so the final (smallest) block's
    gather+wo tail is minimal.
  - PSUM: scores 4 banks (2 bufs x [128,1024]), PV accumulators 2 banks,
    shared proj/wo/transpose pool 2 banks = 8.
All HBM->SBUF loads use host-prearranged per-partition-contiguous layouts.
Compute dtype: bf16 matmul operands, fp32 PSUM accumulation, fp32 softmax.
"""

import numpy as np

B, S, D = 2, 2048, 2048
H, KVH, HD = 32, 8, 64
NCORES = 8
BS = B * S            # 4096
NHL = H // NCORES     # 4 q heads per core
MQ = NHL * HD         # 256 q columns per core
SBLK = 512
NSBLK = BS // SBLK    # 8
DC = D // 128         # 16 contraction chunks
NKCH = S // 128       # 16 key chunks per batch
NQS = S // SBLK       # 4 query blocks per batch


def build_graph(tc, out_ap, xT, wqEO, kvw, woS, c4, s4):
    """Build the per-core SPMD graph. All args are DRAM access patterns."""
    import concourse.mybir as mybir
    from concourse.masks import make_identity

    nc = tc.nc
    F32 = mybir.dt.float32
    BF16 = mybir.dt.bfloat16
    Alu = mybir.AluOpType
    Act = mybir.ActivationFunctionType
    TT = nc.vector.tensor_tensor
    CP = nc.vector.tensor_copy

    with tc.tile_pool(name="const", bufs=1) as constp, \
         tc.tile_pool(name="persist", bufs=1) as pers, \
         tc.tile_pool(name="dram", bufs=1, space="DRAM") as dramp:

        ident = constp.tile([128, 128], F32)
        make_identity(nc, ident[:])
        tri01f = constp.tile([128, 128], F32)
        nc.gpsimd.memset(tri01f[:], 1.0)
        nc.gpsimd.affine_select(
            out=tri01f[:], in_=tri01f[:], compare_op=Alu.is_ge, fill=0.0,
            base=0, channel_multiplier=-1, pattern=[[1, 128]])
        tri01 = constp.tile([128, 128], BF16)
        CP(tri01[:], tri01f[:])

        # weights/trig DMAs, ordered so the first projection chunk can start
        # ~2us in: wq g0, x(sb0), wq g1-3, kvw, trig
        wq_sb = constp.tile([128, DC, MQ], BF16)
        wqr = wqEO.rearrange("p (dc m) -> p dc m", dc=DC)
        nc.sync.dma_start(wq_sb[:, 0:4, :], wqr[:, 0:4, :])

        xTr = xT.rearrange("p (sb dc s) -> p sb dc s", sb=NSBLK, dc=DC)

        with tc.tile_pool(name="trig", bufs=1) as trigp, \
             tc.tile_pool(name="xtp", bufs=6) as xtp, \
             tc.tile_pool(name="ropep", bufs=2) as rp, \
             tc.tile_pool(name="ptp", bufs=3) as ptp, \
             tc.tile_pool(name="recp", bufs=2) as recp, \
             tc.tile_pool(name="wop", bufs=1) as wop, \
             tc.tile_pool(name="attsp", bufs=3) as attsp, \
             tc.tile_pool(name="outsp", bufs=2) as outsp:
            psA = psS = psO = psX = None  # PSUM pools, scoped in the driver

            def xt_dmas(sb):
                xts = []
                for g in range(4):
                    xt = xtp.tile([128, 4, SBLK], BF16, tag="xt", name=f"xt{g}")
                    nc.sync.dma_start(xt[:], xTr[:, sb, 4 * g:4 * g + 4, :])
                    xts.append(xt)
                return xts

            xts0 = xt_dmas(0)
            for g in range(1, 4):
                nc.sync.dma_start(wq_sb[:, 4 * g:4 * g + 4, :],
                                  wqr[:, 4 * g:4 * g + 4, :])
            # secondary weights/trig go via the gpsimd (SWDGE) queue so the
            # sync queue streams wq+x at full rate from t=0
            kvw_sb = constp.tile([128, DC, 128], BF16)
            nc.gpsimd.dma_start(kvw_sb[:], kvw.rearrange("p (dc m) -> p dc m", dc=DC))
            c4_sb = trigp.tile([128, S], F32)
            nc.gpsimd.dma_start(c4_sb[:], c4[:])
            s4_sb = trigp.tile([128, S], F32)
            nc.gpsimd.dma_start(s4_sb[:], s4[:])
            wo_sb = wop.tile([128, DC, MQ], BF16)
            nc.gpsimd.dma_start(wo_sb[:], woS.rearrange("p (dc m) -> p dc m", dc=DC))

            qt0 = pers.tile([128, BS], BF16)   # heads 0,1 (rows [0:64], [64:128])
            qt1 = pers.tile([128, BS], BF16)   # heads 2,3
            kT2 = pers.tile([128, BS], BF16)   # kT duplicated at base 0 and 64
            v1 = pers.tile([128, B * NKCH, 128], BF16)  # [v | ones] per chunk
            attnT0 = pers.tile([128, BS], BF16)
            attnT1 = pers.tile([128, BS], BF16)
            qts = [qt0, qt1]
            attnTs = [attnT0, attnT1]
            nc.gpsimd.memset(v1[:, :, 64:128], 1.0)

            def proj_sb(sb, xts, on_act, P):
                """Generator: projection+RoPE for one 512-col block of x^T.
                Yields ~1us-of-PE chunks. on_act: route copies to the scalar
                engine (idle outside attention) instead of DVE. P(tag) is the
                PSUM allocator (dedicated pool for b0, shared psX for b1)."""
                scol = slice(sb * SBLK, (sb + 1) * SBLK)
                pbeg = (sb % NQS) * SBLK
                qE_p = P("qE")
                for dc in range(DC):
                    nc.tensor.matmul(qE_p[:], wq_sb[:, dc, 0:128],
                                     xts[dc // 4][:, dc % 4],
                                     start=(dc == 0), stop=(dc == DC - 1))
                    if dc % 4 == 3:
                        yield
                qO_p = P("qO")
                for dc in range(DC):
                    nc.tensor.matmul(qO_p[:], wq_sb[:, dc, 128:256],
                                     xts[dc // 4][:, dc % 4],
                                     start=(dc == 0), stop=(dc == DC - 1))
                    if dc % 4 == 3:
                        yield
                c_ = c4_sb[:, pbeg:pbeg + SBLK]
                s_ = s4_sb[:, pbeg:pbeg + SBLK]
                # q RoPE: qE_p rows = 4 heads x 32 even comps, qO_p odd comps
                m1 = rp.tile([128, SBLK], F32, tag="m1")
                TT(m1[:], qE_p[:], c_, Alu.mult)
                m2 = rp.tile([128, SBLK], F32, tag="m2")
                TT(m2[:], qO_p[:], s_, Alu.mult)
                m3 = rp.tile([128, SBLK], F32, tag="m3")
                TT(m3[:], qO_p[:], c_, Alu.mult)
                m4 = rp.tile([128, SBLK], F32, tag="m4")
                TT(m4[:], qE_p[:], s_, Alu.mult)
                oE = rp.tile([128, SBLK], BF16, tag="oE")
                TT(oE[:], m1[:], m2[:], Alu.subtract)
                oO = rp.tile([128, SBLK], BF16, tag="oO")
                TT(oO[:], m3[:], m4[:], Alu.add)
                kv_p = P("kv")
                for dc in range(DC):
                    nc.tensor.matmul(kv_p[:], kvw_sb[:, dc, :],
                                     xts[dc // 4][:, dc % 4],
                                     start=(dc == 0), stop=(dc == DC - 1))
                    if dc % 4 == 3:
                        yield
                cp = nc.scalar.copy if on_act else CP
                # v first: evacuating vT promptly unblocks this PSUM slot for
                # the next block's chain (it's the slot-gating reader)
                vtw = rp.tile([64, SBLK], F32, tag="vtw")
                cp(vtw[:], kv_p[64:128, :])
                # k RoPE: kv_p rows [0:32]=kE, [32:64]=kO, [64:128]=vT
                a1 = rp.tile([32, SBLK], F32, tag="a1")
                TT(a1[:], kv_p[0:32, :], c_[0:32, :], Alu.mult)
                b1 = rp.tile([32, SBLK], F32, tag="b1")
                TT(b1[:], kv_p[32:64, :], s_[0:32, :], Alu.mult)
                a2 = rp.tile([32, SBLK], F32, tag="a2")
                TT(a2[:], kv_p[32:64, :], c_[0:32, :], Alu.mult)
                b2 = rp.tile([32, SBLK], F32, tag="b2")
                TT(b2[:], kv_p[0:32, :], s_[0:32, :], Alu.mult)
                TT(kT2[0:32, scol], a1[:], b1[:], Alu.subtract)
                TT(kT2[32:64, scol], a2[:], b2[:], Alu.add)
                yield
                tpx = P("tp")
                for q in range(4):
                    nc.tensor.transpose(tpx[:, 64 * q:64 * q + 64],
                                        vtw[:, q * 128:(q + 1) * 128],
                                        ident[0:64, 0:64])
                ch = sb * 4
                CP(v1[:, ch:ch + 4, 0:64],
                   tpx[:, 0:256].rearrange("p (c f) -> p c f", c=4))
                yield
                for h in range(NHL):
                    t, j = h // 2, h % 2
                    cp(qts[t][64 * j:64 * j + 32, scol], oE[32 * h:32 * h + 32, :])
                    cp(qts[t][64 * j + 32:64 * j + 64, scol], oO[32 * h:32 * h + 32, :])
                yield
                cp(kT2[64:96, scol], kT2[0:32, scol])
                cp(kT2[96:128, scol], kT2[32:64, scol])
                yield

            def proj_stream(sbs, on_act, P):
                """Chain proj generators with 1-block xt DMA lookahead."""
                pending = {sb: None for sb in sbs}
                pending[sbs[0]] = xts0 if sbs[0] == 0 else xt_dmas(sbs[0])
                for idx, sb in enumerate(sbs):
                    if idx + 1 < len(sbs):
                        pending[sbs[idx + 1]] = xt_dmas(sbs[idx + 1])
                    yield from proj_sb(sb, pending[sb], on_act, P)

            # ---- attention output segments: one per (b, qsub); the last
            # seg is gathered per head-pair so its t=0 half ships while t=1
            # attention still runs ----
            attnT_loc = [dramp.tile([MQ, SBLK], BF16, name=f"attnT_loc{i}")
                         for i in range(8)]
            attnT_all = [dramp.tile([D, SBLK], BF16, addr_space="Shared",
                                    name=f"attnT_all{i}") for i in range(8)]
            attnT_loc7 = [dramp.tile([128, SBLK], BF16, name=f"attnT_loc7{t}")
                          for t in range(2)]
            attnT_all7 = [dramp.tile([D // 2, SBLK], BF16, addr_space="Shared",
                                     name=f"attnT_all7{t}") for t in range(2)]

            def wo_dmas(seg):
                # gather-gated loads live on the gpsimd queue: on the sync
                # queue the scheduler hoists them ahead of xt loads and the
                # collective wait head-blocks the whole bulk-load stream
                attr = attnT_all[seg].rearrange("(dc p) s -> p dc s", p=128)
                atts = []
                for hf in range(2):
                    at = attsp.tile([128, DC // 2, SBLK], BF16, tag="att",
                                    name=f"att{hf}")
                    nc.gpsimd.dma_start(at[:], attr[:, 8 * hf:8 * hf + 8, :])
                    atts.append(at)
                return atts

            def wo_seg(seg, atts):
                """Generator: wo matmuls for one gathered 512-col segment."""
                b, qs = seg // NQS, seg % NQS
                for mc in range(2):
                    wp = psX.tile([128, SBLK], F32, tag="x", name="wp")
                    for dc in range(DC):
                        nc.tensor.matmul(
                            wp[:], wo_sb[:, dc, mc * 128:(mc + 1) * 128],
                            atts[dc // 8][:, dc % 8, :],
                            start=(dc == 0), stop=(dc == DC - 1))
                        if dc % 4 == 3:
                            yield
                    ot = outsp.tile([128, SBLK], F32, tag="ot")
                    CP(ot[:], wp[:])
                    nc.sync.dma_start(
                        out_ap[mc * 128:(mc + 1) * 128,
                               b * S + qs * SBLK:b * S + (qs + 1) * SBLK],
                        ot[:])
                    yield

            gathered_at = {}   # seg -> fill counter when its gather emitted
            fills = [0]        # shared fill-slot counter
            MARGIN = 12        # fill slots (~23us) for the gather to execute

            def wo_att7_load(t):
                attr = attnT_all7[t].rearrange("(c p) s -> p c s", p=128)
                at = attsp.tile([128, NCORES, SBLK], BF16, tag="att",
                                name=f"att7{t}")
                nc.gpsimd.dma_start(at[:], attr[:])
                return at

            def wo_seg7_half(t, at, wps):
                """One head-pair half of the final wo segment. t=0 opens the
                PSUM accumulation (runs while the t=1 gather is in flight);
                t=1 closes it and ships the output."""
                b, qs = 1, 3
                for mc in range(2):
                    if t == 0:
                        wps.append(psX.tile([128, SBLK], F32, tag="x",
                                            name="wp"))
                    wp = wps[mc]
                    for c in range(NCORES):
                        nc.tensor.matmul(
                            wp[:], wo_sb[:, 2 * c + t, mc * 128:(mc + 1) * 128],
                            at[:, c, :],
                            start=(t == 0 and c == 0),
                            stop=(t == 1 and c == NCORES - 1))
                    if t == 1:
                        ot = outsp.tile([128, SBLK], F32, tag="ot")
                        CP(ot[:], wp[:])
                        nc.sync.dma_start(
                            out_ap[mc * 128:(mc + 1) * 128,
                                   b * S + qs * SBLK:b * S + (qs + 1) * SBLK],
                            ot[:])

            def wo_stream(segs, preloaded=None):
                """wo segments with att-tile DMA prefetch one seg ahead,
                gated on the seg's gather having been emitted MARGIN fill
                slots ago (so the collective has likely completed)."""
                atts = dict(preloaded or {})

                def ready(s):
                    return s in gathered_at and \
                        fills[0] >= gathered_at[s] + MARGIN

                def ensure(s):
                    if s is not None and s not in atts and ready(s):
                        atts[s] = wo_dmas(s)

                def gen():
                    # let attention run ahead while the first att tiles land
                    for _ in range(10):
                        yield
                    for idx, seg in enumerate(segs):
                        nxt = segs[idx + 1] if idx + 1 < len(segs) else None
                        while not ready(seg):
                            yield
                        ensure(seg)
                        for _ in wo_seg(seg, atts.pop(seg)):
                            ensure(nxt)
                            yield
                return gen()

            def fill(gen):
                fills[0] += 1
                if gen is not None:
                    next(gen, None)

            rA, rB = slice(0, 64), slice(64, 128)

            def attn_batch(b, filler, fill_every, qs_order):
                """Attention for batch b, pulling filler chunks into the PE
                stream between iterations."""
                it = 0
                pend = []
                for qs in qs_order:
                    qcg = slice(b * S + qs * 512, b * S + (qs + 1) * 512)
                    for t in range(2):
                        oP = [psO.tile([128, SBLK], F32, tag=f"o{i}",
                                       name=f"o{i}") for i in range(2)]
                        # diagonal chunks first: the PV accumulation then
                        # starts and stops on full-width matmuls, with the
                        # narrowed diag writes in the middle
                        ks = list(range(4 * qs, 4 * qs + 4)) + \
                            list(range(0, 4 * qs))
                        for ki, k in enumerate(ks):
                            diag = (k // 4) == qs
                            es = 128 * (k % 4) if diag else 0
                            full_pv = qs == 0  # all-diag chain: pad with 0s
                            kc = slice(b * S + k * 128, b * S + k * 128 + 128)
                            qc = slice(b * S + qs * 512 + es,
                                       b * S + qs * 512 + 512)
                            sP = psS.tile([128, 1024], F32, tag="s", name="s")
                            nc.tensor.matmul(sP[:, es:512], kT2[rA, kc],
                                             qts[t][rA, qc],
                                             start=True, stop=True)
                            nc.tensor.matmul(sP[:, 512 + es:1024], kT2[rB, kc],
                                             qts[t][rB, qc],
                                             start=True, stop=True)
                            pP = ptp.tile([128, 1024], BF16, tag="p", name="p")
                            if es and full_pv:
                                nc.vector.memset(pP[:, 0:es], 0.0)
                                nc.vector.memset(pP[:, 512:512 + es], 0.0)
                            if es:
                                nc.scalar.activation(
                                    pP[:, es:512], sP[:, es:512],
                                    Act.Exp, scale=0.125)
                                nc.scalar.activation(
                                    pP[:, 512 + es:1024], sP[:, 512 + es:1024],
                                    Act.Exp, scale=0.125)
                            else:
                                nc.scalar.activation(
                                    pP[:], sP[:], Act.Exp, scale=0.125)
                            if diag:
                                # zero the causal triangle (key > q)
                                TT(pP[:, es:es + 128], pP[:, es:es + 128],
                                   tri01[:], Alu.mult)
                                TT(pP[:, 512 + es:512 + es + 128],
                                   pP[:, 512 + es:512 + es + 128],
                                   tri01[:], Alu.mult)
                            it += 1
                            if it % fill_every == 0:
                                fill(filler)
                            if pend:
                                pend.pop()()
                            def pv(pP=pP, k=k, ki=ki,
                                   es=(0 if full_pv else es)):
                                for i in range(2):
                                    nc.tensor.matmul(
                                        oP[i][:, es:512],
                                        v1[:, b * NKCH + k, :],
                                        pP[:, 512 * i + es:512 * i + 512],
                                        start=(ki == 0),
                                        stop=(ki == 4 * qs + 3),
                                        skip_group_check=(es != 0))
                            pend.append(pv)
                        if pend:
                            pend.pop()()
                        # normalize: oP rows [64:128] hold the denominator
                        for i, rows in enumerate((rA, rB)):
                            raw = recp.tile([128, SBLK], F32, tag=f"raw{i}",
                                            name=f"raw{i}")
                            CP(raw[:], oP[i][:])
                            den = recp.tile([64, SBLK], F32, tag=f"den{i}",
                                            name=f"den{i}")
                            CP(den[:], raw[64:128, :])
                            rec = recp.tile([64, SBLK], F32, tag=f"rec{i}",
                                            name=f"rec{i}")
                            nc.vector.reciprocal_approx_fast(rec[:], den[:])
                            TT(attnTs[t][rows, qcg], raw[0:64, :], rec[:],
                               Alu.mult)
                        if b * NQS + qs == 7:
                            nc.sync.dma_start(attnT_loc7[t][:],
                                              attnTs[t][:, qcg])
                            nc.gpsimd.collective_compute(
                                "AllGather", mybir.AluOpType.bypass,
                                replica_groups=[list(range(NCORES))],
                                ins=[attnT_loc7[t].opt()],
                                outs=[attnT_all7[t].opt()])
                            gathered_at[(7, t)] = fills[0]
                        fill(filler)
                    seg = b * NQS + qs
                    if seg < 7:
                        nc.sync.dma_start(attnT_loc[seg][0:128, :],
                                          attnTs[0][:, qcg])
                        nc.sync.dma_start(attnT_loc[seg][128:256, :],
                                          attnTs[1][:, qcg])
                        nc.gpsimd.collective_compute(
                            "AllGather", mybir.AluOpType.bypass,
                            replica_groups=[list(range(NCORES))],
                            ins=[attnT_loc[seg].opt()],
                            outs=[attnT_all[seg].opt()])
                        gathered_at[seg] = fills[0]

            # ---- the pipelined program ----
            def PA(tag):
                return psA.tile([128, SBLK], F32, tag=tag, name=tag)

            def PX(tag):
                return psX.tile([128, SBLK], F32, tag="x", name=tag)

            with tc.tile_pool(name="psA", bufs=2, space="PSUM") as psA:
                projA = proj_stream([0, 1, 2, 3], True, PA)
                for _ in projA:
                    pass
            with tc.tile_pool(name="psS", bufs=2, space="PSUM") as psS, \
                 tc.tile_pool(name="psO", bufs=1, space="PSUM") as psO, \
                 tc.tile_pool(name="psX", bufs=2, space="PSUM") as psX:
                import itertools
                projB = proj_stream([4, 5, 6, 7], False, PX)

                # b0's attention gets b1-proj blocks 4-6 only. Block 7 (keys
                # 12-15 / queries of b1-qs3, which runs LAST in b1) fills
                # b1's early attention instead -- ungated work exactly where
                # the gather-paced dead zones are. All wo goes to b1 after
                # it; seg order matches gather production (b1 runs qs
                # [2,0,1,3] -> segs 6,4,5 then the split 7).
                attn_batch(0, itertools.islice(projB, 48), 1, [0, 1, 2, 3])
                pre = {s: wo_dmas(s) for s in (0, 1)}
                woA = wo_stream([0, 1, 2, 3, 6, 4, 5], pre)
                fillerB = itertools.chain(projB, woA)
                # qs2 first: its gather (the would-be tail straggler) fires
                # early; qs3's split gather halves are then the only tail work
                attn_batch(1, fillerB, 1, [2, 0, 1, 3])
                fills[0] = 1 << 30   # drain: no more fill slots, no gating
                for _ in fillerB:
                    pass
                # final segment: the t=0 half opens the PSUM accumulation as
                # soon as its gather lands, overlapping the t=1 gather wait
                wps = []
                wo_seg7_half(0, wo_att7_load(0), wps)
                wo_seg7_half(1, wo_att7_load(1), wps)


def prep_inputs(x, cos, sin, wq, wk, wv, wo):
    """Host-side layout prep. Returns per-core input dicts (bf16/f32).

    All SBUF-bound tensors are prearranged so that each of the 128 SBUF
    partitions reads one contiguous DRAM chunk (fat DMA descriptors).
    """
    import ml_dtypes
    bf16 = ml_dtypes.bfloat16
    x = np.asarray(x, np.float32)
    cos = np.asarray(cos, np.float32)
    sin = np.asarray(sin, np.float32)
    wq = np.asarray(wq, np.float32)
    wk = np.asarray(wk, np.float32)
    wv = np.asarray(wv, np.float32)
    wo = np.asarray(wo, np.float32)

    xT = np.ascontiguousarray(x.transpose(2, 0, 1).reshape(D, BS)).astype(bf16)
    # [dc*128+p, sb*512+s] -> [p, sb, dc, s] flattened per partition
    xTc = np.ascontiguousarray(
        xT.reshape(DC, 128, NSBLK, SBLK).transpose(1, 2, 0, 3).reshape(128, -1))
    c4 = np.ascontiguousarray(np.tile(cos.T, (4, 1)))          # [128, S] f32
    s4 = np.ascontiguousarray(np.tile(sin.T, (4, 1)))
    eperm = np.array([64 * h + 2 * j for h in range(NHL) for j in range(32)])
    operm = eperm + 1

    def pmajor(w):  # [D, M] -> [128, DC*M] with [p, dc, m] contiguous
        m = w.shape[1]
        return np.ascontiguousarray(
            w.reshape(DC, 128, m).transpose(1, 0, 2).reshape(128, -1)).astype(bf16)

    in_maps = []
    for c in range(NCORES):
        wq_sh = wq[:, MQ * c:MQ * c + MQ]
        wqEO = np.concatenate([wq_sh[:, eperm], wq_sh[:, operm]], axis=1)
        kc = wk[:, HD * c:HD * c + HD]
        vc = wv[:, HD * c:HD * c + HD]
        kvw = np.concatenate([kc[:, 0::2], kc[:, 1::2], vc], axis=1)
        woS = wo[:, MQ * c:MQ * c + MQ]
        in_maps.append({
            "xT": xTc,
            "wqEO": pmajor(wqEO),
            "kvw": pmajor(kvw),
            "woS": pmajor(woS),
            "c4": c4,
            "s4": s4,
        })
    return in_maps


def assemble_output(core_outs):
    """core_outs: list of 8 [256, BS] f32 arrays -> [B, S, D] f32."""
    outT = np.concatenate(core_outs, axis=0)           # [D, BS]
    return np.ascontiguousarray(
        outT.reshape(D, B, S).transpose(1, 2, 0)).astype(np.float32)


_CACHE = {}


def _get_compiled():
    if "nc" in _CACHE:
        return _CACHE["nc"]
    import concourse.mybir as mybir
    import concourse.tile as tile
    from concourse import bacc

    nc = bacc.Bacc("TRN2", target_bir_lowering=False, debug=False,
                   num_devices=NCORES)
    F32 = mybir.dt.float32
    BF16 = mybir.dt.bfloat16
    xT_d = nc.dram_tensor("xT", [128, NSBLK * DC * SBLK], BF16, kind="ExternalInput")
    wq_d = nc.dram_tensor("wqEO", [128, DC * MQ], BF16, kind="ExternalInput")
    kvw_d = nc.dram_tensor("kvw", [128, DC * 128], BF16, kind="ExternalInput")
    wo_d = nc.dram_tensor("woS", [128, DC * MQ], BF16, kind="ExternalInput")
    c4_d = nc.dram_tensor("c4", [128, S], F32, kind="ExternalInput")
    s4_d = nc.dram_tensor("s4", [128, S], F32, kind="ExternalInput")
    out_d = nc.dram_tensor("out", [MQ, BS], F32, kind="ExternalOutput")
    with tile.TileContext(nc) as tc:
        build_graph(tc, out_d.ap(), xT_d.ap(), wq_d.ap(), kvw_d.ap(),
                    wo_d.ap(), c4_d.ap(), s4_d.ap())
    nc.compile()
    _CACHE["nc"] = nc
    return nc


def kernel(x, cos, sin, wq, wk, wv, wo):
    from concourse.bass_utils import run_bass_kernel_spmd
    nc = _get_compiled()
    in_maps = prep_inputs(x, cos, sin, wq, wk, wv, wo)
    res = run_bass_kernel_spmd(nc, in_maps, core_ids=list(range(NCORES)))
    _CACHE["last_results"] = res
    return assemble_output([res.results[c]["out"] for c in range(NCORES)])

